# revision 14
# baseline (speedup 1.0000x reference)
"""BiLSTM-CRF NLL kernel for 8 TRN2 NeuronCores (v2).

Sharding: data-parallel over batch. B=128 split into 8 shards of 16
sentences; each core runs both LSTM directions, the fc projection, the
CRF forward pass and the gold-path score for its shard.

v2 design (vs baseline):
  - W_ih folded into the embedding table on the host:
    preW[v] = emb[v] @ W_ih^T + (b_ih + b_hh), bf16, rows permuted to
    [i|f|o|g] with the g block pre-scaled by 2 (tanh(g) = 2*sigmoid(2g)-1).
    The per-step input contribution is a single indirect row gather +
    16 PE transposes + one DVE copy per 8-step window per direction.
  - Gates accumulate in PSUM: identity-matmul injects the pre slice,
    then 64 bf16 W_hh matmuls accumulate on top (start=False). No
    separate pre-add on the elementwise chain.
  - Per-step elementwise (per dir): 2 sigmoids (gi, fo views), fused
    tanh(g) via tensor_scalar 2s-1, 4-5 DVE tensor ops, 1 tanh.
    Forward dir runs unmasked (post-length values unused); backward
    keeps masked state in cst/hcurb via copy_predicated.
  - CRF: stationary matrix augmented to [12,13] with an all-ones column
    so every step's matmul also yields the column sum (for renorm)
    for free; renormalization is applied two epochs late off the
    critical chain; per-step constant e^-2.5 damping (compensated by
    +2.5*len at the end) keeps magnitudes in f32 range.
"""

import os
import numpy as np
import ml_dtypes

import concourse.bass as bass
import concourse.bacc as bacc
import concourse.mybir as mybir
import concourse.tile as tile
from concourse.bass import AP
from concourse.masks import make_identity

F32 = mybir.dt.float32
BF16 = mybir.dt.bfloat16
I32 = mybir.dt.int32
U8 = mybir.dt.uint8
MUL = mybir.AluOpType.mult
ADD = mybir.AluOpType.add
SUB = mybir.AluOpType.subtract
X = mybir.AxisListType.X
SIG = mybir.ActivationFunctionType.Sigmoid
TANH = mybir.ActivationFunctionType.Tanh
EXP = mybir.ActivationFunctionType.Exp
LN = mybir.ActivationFunctionType.Ln

P = 128
B = 16            # batch per core
H = 512
G = 2048          # 4H
K = 12
START, STOP = 10, 11
R = 8             # CRF renorm epoch length
W = 8             # pre-gather window (steps per indirect gather)
NCORES = 8
SHIFT = 2.5       # per-step CRF damping exp(-SHIFT)

T = int(os.environ.get("BASS_LSTM_T", "256"))
SKIP = set(os.environ.get("BASS_SKIP", "").split(","))
NW = T // W       # number of gather windows
NJ = T // R - 1   # number of CRF renorm epochs with a recorded sum


def fv(t, off, pat):
    """Free-dim view of a contiguous [P, F] tile: keep partition pair, replace
    free dims with `pat` (list of [step, count]) at element offset `off`."""
    base = t[:] if not isinstance(t, AP) else t
    part = list(base.ap[0])
    return AP(base.tensor, base.offset + off, [part] + [list(p) for p in pat])


def build(nc):
    dirs = ("f", "b")
    dt = {}

    def din(name, shape, dtype):
        dt[name] = nc.dram_tensor(name, shape, dtype, kind="ExternalInput")
        return dt[name]

    for d in dirs:
        din(f"xw_{d}", [T * B], I32)
        din(f"preW_{d}", [30000, G], BF16)
        din(f"whhT_{d}", [H, G], BF16)
        din(f"h0T_{d}", [P, 64], BF16)
        din(f"c0T_{d}", [P, 64], F32)
        din(f"fcWT_{d}", [H, K], BF16)
    din("mask_b", [T, P, 64], U8)
    din("transT", [K, K], F32)
    din("trans", [K, K], F32)
    din("fcb", [K], F32)
    din("a0", [K, B], F32)
    din("msel", [K, T * B], F32)
    din("mprefix", [NJ * B], F32)
    din("lenc", [B], F32)
    din("sel", [K, T * B], F32)
    din("counts", [B, 144], F32)
    din("cntb", [B, K], F32)

    nll_o = nc.dram_tensor("nll", [B], F32, kind="ExternalOutput")
    demis_o = nc.dram_tensor("dbg_emis", [K, T * B], F32, kind="ExternalOutput")
    dlogz_o = nc.dram_tensor("dbg_logz", [B], F32, kind="ExternalOutput")
    dgold_o = nc.dram_tensor("dbg_gold", [B], F32, kind="ExternalOutput")
    dhs_o = None
    if os.environ.get("BASS_DBG"):
        dhs_o = {d: nc.dram_tensor(f"dbg_hs_{d}", [P, (T + 1) * 64], BF16,
                                   kind="ExternalOutput") for d in ("f", "b")}
    dg0_o = None
    if os.environ.get("BASS_DBG"):
        dg0_o = nc.dram_tensor("dbg_g0", [P, 512], F32, kind="ExternalOutput")
        dpre_o = nc.dram_tensor("dbg_pre0", [P, G], BF16, kind="ExternalOutput")

    scr16 = nc.dram_tensor("scr16", [B], F32)

    with tile.TileContext(nc) as tc:
        with tc.tile_pool(name="persist", bufs=1) as pp:
            whh = {d: pp.tile([P, 4 * 16 * P], BF16, name=f"whh{d}", tag=f"whh{d}")
                   for d in dirs}
            fcw = {d: pp.tile([P, 4 * K], BF16, name=f"fcw{d}", tag=f"fcw{d}") for d in dirs}
            hs = {d: pp.tile([P, (T + 1) * 64], BF16, name=f"hs{d}", tag=f"hs{d}")
                  for d in dirs}
            cst = {d: pp.tile([P, 64], F32, name=f"cst{d}", tag=f"c{d}") for d in dirs}
            hcurb = pp.tile([P, 64], BF16, tag="hcurb")
            identB = pp.tile([P, P], BF16, tag="identB")
            emisT = pp.tile([K, T * B], F32, tag="emisT")
            hist = pp.tile([K, T * B], F32, tag="hist")
            expem = pp.tile([K, T * B], F32, tag="expem")
            Sall = pp.tile([1, (NJ + 1) * B], F32, tag="Sall")
            idxall = {d: pp.tile([P, NW], I32, name=f"idxall{d}", tag=f"idxall{d}")
                      for d in dirs}

            make_identity(nc, identB[:])
            nc.gpsimd.memset(Sall[:], 1.0)
            for d in dirs:
                for k in range(4):
                    nc.gpsimd.dma_start(
                        whh[d][:, k * 16 * P:(k + 1) * 16 * P],
                        dt[f"whhT_{d}"].ap()[k * P:(k + 1) * P, :])
                    nc.gpsimd.dma_start(
                        fcw[d][:, k * K:(k + 1) * K],
                        dt[f"fcWT_{d}"].ap()[k * P:(k + 1) * P, :])
                nc.gpsimd.dma_start(hs[d][:, 0:64], dt[f"h0T_{d}"].ap()[:])
                nc.gpsimd.dma_start(cst[d][:], dt[f"c0T_{d}"].ap()[:])
                nc.gpsimd.dma_start(
                    idxall[d][:], AP(dt[f"xw_{d}"], 0, [[1, P], [P, NW]]))
            nc.gpsimd.dma_start(hcurb[:], dt["h0T_b"].ap()[:])

            # ---- recurrence with inlined pre-staging ----
            with tc.tile_pool(name="rec_sbuf", bufs=2) as rp, \
                 tc.tile_pool(name="stage_psum", bufs=1, space="PSUM") as stp, \
                 tc.tile_pool(name="gate_psum", bufs=3, space="PSUM") as gpp:

                prechW = {}
                maskch = None

                def stage(w, d):
                    rows = rp.tile([P, G], BF16, name=f"rows{d}", tag=f"rows{d}")
                    nc.gpsimd.indirect_dma_start(
                        out=rows[:], out_offset=None,
                        in_=dt[f"preW_{d}"].ap()[:],
                        in_offset=bass.IndirectOffsetOnAxis(
                            ap=idxall[d][:, w:w + 1], axis=0))
                    stg = stp.tile([P, G], BF16, name=f"stg{d}", tag="stg")
                    for m in range(16):
                        nc.tensor.transpose(
                            stg[:, m * P:(m + 1) * P], rows[:, m * P:(m + 1) * P],
                            identB[:])
                    pc = rp.tile([P, G], BF16, name=f"prech{d}", tag=f"prech{d}")
                    nc.vector.tensor_copy(pc[:], stg[:])
                    return pc

                def load_mask(w):
                    mk = rp.tile([P, W * 64], U8, tag="maskch")
                    nc.gpsimd.dma_start(
                        mk[:], AP(dt["mask_b"], w * W * P * 64,
                                  [[64, P], [P * 64, W], [1, 64]]))
                    return mk

                if "rec" not in SKIP:
                    for d in dirs:
                        prechW[d] = stage(0, d)
                    maskch = load_mask(0)
                nextprech = {}

                # m-order: g block first, then i, then f,o — lets sigma(g,i)
                # start while the f,o matmuls still stream.
                m_order = [12, 13, 14, 15, 0, 1, 2, 3, 4, 5, 6, 7, 8, 9, 10, 11]

                rec_range = range(0, T) if "rec" not in SKIP else range(0)
                for t in rec_range:
                    w, tl = t // W, t % W
                    if tl == 0 and w + 1 < NW:
                        for d in dirs:
                            nextprech[d] = stage(w + 1, d)
                        nextmask = load_mask(w + 1)
                    psd = {d: gpp.tile([P, 256], F32, name=f"gates{d}",
                                       tag=f"gates{d}") for d in dirs}
                    for d in dirs:
                        nc.tensor.matmul(
                            psd[d][:], identB[:],
                            fv(prechW[d], tl * B, [[P, 16], [1, B]]),
                            start=True, stop=False, skip_group_check=True)
                    for d in dirs:
                        ps = psd[d]
                        do = 0
                        for mi, m in enumerate(m_order):
                            for k in range(4):
                                if d == "f":
                                    rhs = hs[d][:, t * 64 + k * B: t * 64 + (k + 1) * B]
                                else:
                                    rhs = hcurb[:, k * B:(k + 1) * B]
                                nc.tensor.matmul(
                                    ps[:, m * B:(m + 1) * B],
                                    whh[d][:, (k * 16 + m) * P:(k * 16 + m + 1) * P],
                                    rhs, start=False,
                                    stop=(mi == 15 and k == 3),
                                    skip_group_check=True)
                        if t == 0 and d == "f" and dg0_o is not None:
                            gev = rp.tile([P, 256], F32, tag="gev")
                            nc.vector.tensor_copy(gev[:], ps[:])
                            nc.gpsimd.dma_start(AP(dg0_o, 0, [[512, P], [1, 256]]), gev[:])
                            pev = rp.tile([P, G], BF16, tag="pev")
                            nc.vector.tensor_copy(pev[:], prechW["f"][:])
                            nc.gpsimd.dma_start(dpre_o.ap()[:], pev[:])
                        # elementwise chain. sigma split into (g,i) and (f,o)
                        # halves so sigma(g,i) overlaps the f,o matmuls; the
                        # cell update is regrouped as cn = t1 + (u - sigma_i)
                        # so only one DVE op trails the late sigma(f,o).
                        sf = rp.tile([P, 256], F32, name=f"sifo{d}", tag=f"sifo{d}")
                        nc.scalar.activation(
                            fv(sf, 0, [[192, 2], [1, 64]]),
                            fv(ps, 0, [[192, 2], [1, 64]]), SIG)
                        nc.scalar.activation(sf[:, 64:192], ps[:, 64:192], SIG)
                        u_ = rp.tile([P, 64], F32, tag=f"u{d}")
                        nc.vector.scalar_tensor_tensor(
                            u_[:], sf[:, 192:256], 2.0, sf[:, 0:64],
                            op0=MUL, op1=MUL)
                        w2_ = rp.tile([P, 64], F32, tag=f"w2{d}")
                        nc.vector.scalar_tensor_tensor(
                            w2_[:], u_[:], 0.0, sf[:, 0:64], op0=SUB, op1=SUB)
                        t1_ = rp.tile([P, 64], F32, tag=f"t1{d}")
                        nc.vector.scalar_tensor_tensor(
                            t1_[:], cst[d][:], 1.0, sf[:, 64:128], op0=MUL, op1=MUL)
                        if d == "f":
                            nc.vector.scalar_tensor_tensor(
                                cst[d][:], t1_[:], 0.0, w2_[:], op0=ADD, op1=ADD)
                            cnsrc = cst[d][:]
                        else:
                            cnb = rp.tile([P, 64], F32, tag="cnb")
                            nc.vector.scalar_tensor_tensor(
                                cnb[:], t1_[:], 0.0, w2_[:], op0=ADD, op1=ADD)
                            nc.vector.copy_predicated(
                                cst[d][:], maskch[:, tl * 64:(tl + 1) * 64], cnb[:])
                            cnsrc = cnb[:]
                        tc_ = rp.tile([P, 64], F32, tag=f"tc{d}")
                        nc.scalar.activation(tc_[:], cnsrc, TANH)
                        hslot = hs[d][:, (t + 1) * 64:(t + 2) * 64]
                        nc.vector.scalar_tensor_tensor(
                            hslot, sf[:, 128:192], 1.0, tc_[:], op0=MUL, op1=MUL)
                        if d == "b":
                            nc.vector.copy_predicated(
                                hcurb[:], maskch[:, tl * 64:(tl + 1) * 64], hslot)
                    if tl == W - 1 and w + 1 < NW:
                        maskch = nextmask
                        prechW = dict(nextprech)

            if dhs_o is not None:
                for d in dirs:
                    nc.gpsimd.dma_start(dhs_o[d].ap()[:], hs[d][:])

            # ---- fc + CRF (interleaved) ----
            with tc.tile_pool(name="crf_sbuf", bufs=2) as cp, \
                 tc.tile_pool(name="crf_persist", bufs=1) as cpr, \
                 tc.tile_pool(name="rs_pool", bufs=3) as rsp, \
                 tc.tile_pool(name="fc_psum", bufs=2, space="PSUM") as fpp, \
                 tc.tile_pool(name="crf_psum", bufs=2, space="PSUM") as cpp:
                # [12, 33] stationary: cols 0:12 = exp(trans)^T, col 32 = ones
                # (colsum lands on out partition 32 — partition reads must be
                # 32-aligned per the BIR verifier).
                etA = cpr.tile([K, 33], F32, tag="etA")
                transTs = cpr.tile([K, K], F32, tag="transTs")
                nc.gpsimd.dma_start(transTs[:], dt["transT"].ap()[:])
                nc.gpsimd.memset(etA[:], 0.0)
                nc.scalar.activation(etA[:, 0:K], transTs[:], EXP)
                nc.gpsimd.memset(etA[:, 32:33], 1.0)
                Estop = cpr.tile([K, 1], F32, tag="Estop")
                nc.scalar.activation(Estop[:], transTs[:, STOP:STOP + 1], EXP)
                fcbm = cpr.tile([K, 1], F32, tag="fcbm")
                nc.gpsimd.dma_start(fcbm[:], AP(dt["fcb"], 0, [[1, K], [1, 1]]))
                nc.vector.tensor_scalar(out=fcbm[:], in0=fcbm[:], scalar1=SHIFT,
                                        scalar2=None, op0=SUB)
                a0 = cpr.tile([K, B], F32, tag="a0")
                nc.gpsimd.dma_start(a0[:], dt["a0"].ap()[:])

                NCH = T * B // 512
                rsap = {}

                def fc_chunk(c):
                    psf = fpp.tile([K, 512], F32, tag="psf")
                    for d in dirs:
                        for k in range(4):
                            if d == "f":
                                rhs = fv(hs[d], (c * 32 + 1) * 64 + k * B,
                                         [[64, 32], [1, B]])
                            else:
                                rhs = fv(hs[d], (T - c * 32) * 64 + k * B,
                                         [[-64, 32], [1, B]])
                            nc.tensor.matmul(
                                psf[:], fcw[d][:, k * K:(k + 1) * K], rhs,
                                start=(d == "f" and k == 0),
                                stop=(d == "b" and k == 3))
                    nc.vector.tensor_copy(emisT[:, c * 512:(c + 1) * 512], psf[:])
                    nc.scalar.activation(expem[:, c * 512:(c + 1) * 512],
                                         emisT[:, c * 512:(c + 1) * 512], EXP,
                                         bias=fcbm[:, 0:1])

                for t in range(0 if "crf" not in SKIP else T, T):
                    if t % 32 == 0 and "fc" not in SKIP:
                        fc_chunk(t // 32)
                    doS = (t % R == 0 and t >= R)
                    doApply = (t % R == 0 and t >= 2 * R)
                    j = t // R - 1
                    for hh, (lo, hi) in enumerate(((0, 8), (8, B))):
                        psc = cpp.tile([33, 8], F32, tag=f"psc{hh}", name=f"psc{hh}")
                        if t == 0:
                            rhs = a0[:, lo:hi]
                        else:
                            rhs = hist[:, (t - 1) * B + lo:(t - 1) * B + hi]
                        nc.tensor.matmul(psc[:], etA[:], rhs,
                                         start=True, stop=True)
                        if doS:
                            nc.vector.tensor_copy(
                                Sall[:, j * B + lo:j * B + hi], psc[32:33, :])
                        nc.vector.tensor_tensor(
                            hist[:, t * B + lo:t * B + hi], psc[0:K, :],
                            expem[:, t * B + lo:t * B + hi], op=MUL)
                        if doApply:
                            nc.vector.tensor_tensor(
                                hist[:, t * B + lo:t * B + hi],
                                hist[:, t * B + lo:t * B + hi],
                                rsap[j - 1][:, lo:hi], op=MUL)
                    if doS:
                        rs1 = cp.tile([1, B], F32, tag="rs1")
                        nc.vector.reciprocal(rs1[:], Sall[:, j * B:(j + 1) * B])
                        ra = rsp.tile([K, B], F32, tag="rsap")
                        nc.gpsimd.partition_broadcast(ra[:], rs1[:])
                        rsap[j] = ra

                if "crf" not in SKIP:
                    # capture at t = len-1
                    mselb = cpr.tile([K, T * B], F32, tag="mselb")
                    nc.gpsimd.dma_start(mselb[:], dt["msel"].ap()[:])
                    nc.vector.tensor_tensor(hist[:], hist[:], mselb[:], op=MUL)
                    aend = cp.tile([K, B], F32, tag="aend")
                    nc.vector.tensor_reduce(aend[:], fv(hist, 0, [[1, B], [B, T]]),
                                            axis=X, op=ADD)
                    azs = cp.tile([K, B], F32, tag="azs")
                    nc.vector.tensor_scalar(out=azs[:], in0=aend[:],
                                            scalar1=Estop[:, 0:1], scalar2=None,
                                            op0=MUL)
                    psz = cpp.tile([33, B], F32, tag="psz", bufs=1)
                    nc.tensor.matmul(psz[:], etA[:], azs[:],
                                     start=True, stop=True)
                    logz0 = cp.tile([1, B], F32, tag="logz0")
                    nc.scalar.activation(logz0[:], psz[32:33, :], LN)
                    # renorm compensation: sum_j ln(S_j) * mprefix
                    lnS = cp.tile([1, NJ * B], F32, tag="lnS")
                    nc.scalar.activation(lnS[:], Sall[:, 0:NJ * B], LN)
                    mpf = cp.tile([1, NJ * B], F32, tag="mpf")
                    nc.gpsimd.dma_start(mpf[:], AP(dt["mprefix"], 0,
                                                   [[1, 1], [1, NJ * B]]))
                    nc.vector.tensor_tensor(lnS[:], lnS[:], mpf[:], op=MUL)
                    Lend = cp.tile([1, B], F32, tag="Lend")
                    nc.vector.tensor_reduce(Lend[:], fv(lnS, 0, [[1, B], [B, NJ]]),
                                            axis=X, op=ADD)
                    lencs = cp.tile([1, B], F32, tag="lencs")
                    nc.gpsimd.dma_start(lencs[:], AP(dt["lenc"], 0, [[1, 1], [1, B]]))
                    logzf = cp.tile([1, B], F32, tag="logzf")
                    nc.vector.tensor_tensor(logzf[:], logz0[:], Lend[:], op=ADD)
                    nc.vector.tensor_tensor(logzf[:], logzf[:], lencs[:], op=ADD)
                    nc.gpsimd.dma_start(AP(dlogz_o, 0, [[1, 1], [1, B]]), logzf[:])
                    nc.gpsimd.dma_start(demis_o.ap()[:], emisT[:])

                    # ---- gold score ----
                    tfl = cp.tile([1, 144], F32, tag="tfl")
                    nc.gpsimd.dma_start(tfl[:], AP(dt["trans"], 0, [[1, 1], [1, 144]]))
                    tfb = cp.tile([B, 144], F32, tag="tfb")
                    nc.gpsimd.partition_broadcast(tfb[:], tfl[:])
                    cnts = cp.tile([B, 144], F32, tag="cnts")
                    nc.gpsimd.dma_start(cnts[:], dt["counts"].ap()[:])
                    pr1 = cp.tile([B, 144], F32, tag="pr1")
                    nc.vector.tensor_tensor(pr1[:], cnts[:], tfb[:], op=MUL)
                    g1 = cp.tile([B, 1], F32, tag="g1")
                    nc.vector.tensor_reduce(g1[:], pr1[:], axis=X, op=ADD)
                    fcbr = cp.tile([1, K], F32, tag="fcbr")
                    nc.gpsimd.dma_start(fcbr[:], AP(dt["fcb"], 0, [[1, 1], [1, K]]))
                    fcbb = cp.tile([B, K], F32, tag="fcbb")
                    nc.gpsimd.partition_broadcast(fcbb[:], fcbr[:])
                    cntbs = cp.tile([B, K], F32, tag="cntbs")
                    nc.gpsimd.dma_start(cntbs[:], dt["cntb"].ap()[:])
                    pr2 = cp.tile([B, K], F32, tag="pr2")
                    nc.vector.tensor_tensor(pr2[:], cntbs[:], fcbb[:], op=MUL)
                    g2 = cp.tile([B, 1], F32, tag="g2")
                    nc.vector.tensor_reduce(g2[:], pr2[:], axis=X, op=ADD)
                    g12 = cp.tile([B, 1], F32, tag="g12")
                    nc.vector.tensor_tensor(g12[:], g1[:], g2[:], op=ADD)
                    nc.gpsimd.dma_start(AP(scr16, 0, [[1, B], [1, 1]]), g12[:])
                    g12r = cp.tile([1, B], F32, tag="g12r")
                    nc.gpsimd.dma_start(g12r[:], AP(scr16, 0, [[1, 1], [1, B]]))

                    selb = cpr.tile([K, T * B], F32, tag="selb")
                    nc.gpsimd.dma_start(selb[:], dt["sel"].ap()[:])
                    nc.vector.tensor_tensor(selb[:], emisT[:], selb[:], op=MUL)
                    g3 = cp.tile([K, B], F32, tag="g3")
                    nc.vector.tensor_reduce(g3[:], fv(selb, 0, [[1, B], [B, T]]),
                                            axis=X, op=ADD)
                    psg = cpp.tile([33, B], F32, tag="psg", bufs=1)
                    nc.tensor.matmul(psg[:], etA[:], g3[:],
                                     start=True, stop=True)
                    goldT = cp.tile([1, B], F32, tag="goldT")
                    nc.vector.tensor_tensor(goldT[:], g12r[:], psg[32:33, :], op=ADD)
                    nc.gpsimd.dma_start(AP(dgold_o, 0, [[1, 1], [1, B]]), goldT[:])
                    nllT = cp.tile([1, B], F32, tag="nllT")
                    nc.vector.tensor_tensor(nllT[:], logzf[:], goldT[:], op=SUB)
                    nc.gpsimd.dma_start(AP(nll_o, 0, [[1, 1], [1, B]]), nllT[:])
    return nc


_CACHE = {}


def get_program():
    if "nc" not in _CACHE:
        nc = bacc.Bacc("TRN2", target_bir_lowering=False, debug=False,
                       num_devices=NCORES)
        build(nc)
        nc.compile()
        _CACHE["nc"] = nc
    return _CACHE["nc"]


def perm_ifog(w):
    # [4H, ...] rows i,f,g,o -> i,f,o,g
    return np.concatenate([w[0:512], w[512:1024], w[1536:2048], w[1024:1536]], 0)


def host_prep(inputs):
    f32 = np.float32
    bf = ml_dtypes.bfloat16
    x = np.asarray(inputs["x"]).astype(np.int32)
    lengths = np.asarray(inputs["lengths"]).astype(np.int64)
    tags = np.asarray(inputs["tags"]).astype(np.int64)
    emb = np.asarray(inputs["embedding"], f32)
    trans = np.asarray(inputs["trans"], f32)
    fcW = np.asarray(inputs["fc_W"], f32)
    fcb = np.asarray(inputs["fc_b"], f32)
    h0 = np.asarray(inputs["h0"], f32)
    c0 = np.asarray(inputs["c0"], f32)

    preWd, whhTd = {}, {}
    for d in ("f", "b"):
        wih = perm_ifog(np.asarray(inputs[f"W_ih_{d}"], f32))
        whh = perm_ifog(np.asarray(inputs[f"W_hh_{d}"], f32))
        bi = perm_ifog(np.asarray(inputs[f"b_ih_{d}"], f32)[:, None])[:, 0]
        bh = perm_ifog(np.asarray(inputs[f"b_hh_{d}"], f32)[:, None])[:, 0]
        preW = emb @ wih.T + (bi + bh)[None, :]
        preW[:, 1536:2048] *= 2.0
        preWd[d] = preW.astype(bf)
        whhT = whh.T.copy()
        whhT[:, 1536:2048] *= 2.0
        whhTd[d] = whhT.astype(bf).copy()

    fcWT = {"f": fcW[:, :H].T.astype(bf).copy(), "b": fcW[:, H:].T.astype(bf).copy()}

    maps = []
    for c in range(NCORES):
        bs = slice(c * B, (c + 1) * B)
        xs = x[bs]            # [16, T]
        ln = lengths[bs]      # [16]
        tg = tags[bs]         # [16, T]
        m = {"trans": trans, "transT": trans.T.astype(f32).copy(), "fcb": fcb}
        for d in ("f", "b"):
            xt = xs.T if d == "f" else xs.T[::-1]      # [T, 16]
            m[f"xw_{d}"] = np.ascontiguousarray(xt).reshape(-1).astype(np.int32)
            m[f"preW_{d}"] = preWd[d]
            m[f"whhT_{d}"] = whhTd[d]
            m[f"fcWT_{d}"] = fcWT[d]
            di = 0 if d == "f" else 1
            h0T = h0[di, bs].T.reshape(4, P, B).transpose(1, 0, 2).reshape(P, 64)
            c0T = c0[di, bs].T.reshape(4, P, B).transpose(1, 0, 2).reshape(P, 64)
            m[f"h0T_{d}"] = h0T.astype(bf).copy()
            m[f"c0T_{d}"] = c0T.astype(f32).copy()
        # bwd mask: step s processes tau = T-1-s; valid iff tau < len
        tau = (T - 1 - np.arange(T))[:, None]          # [T, 1]
        mk = (tau < ln[None, :]).astype(f32)           # [T, 16]
        m["mask_b"] = np.broadcast_to(
            mk[:, None, None, :], (T, P, 4, B)).reshape(T, P, 64).astype(np.uint8).copy()
        a0 = np.zeros((K, B), f32); a0[START, :] = 1.0
        m["a0"] = a0
        msel = np.zeros((K, T, B), f32)
        msel[:, ln - 1, np.arange(B)] = 1.0
        m["msel"] = msel.reshape(K, T * B)
        # renorm j applied at step 8j+16; counted iff 8j+16 <= len-1
        jj = np.arange(NJ)[:, None]
        m["mprefix"] = ((R * jj + 2 * R) <= (ln[None, :] - 1)).astype(f32).reshape(-1)
        m["lenc"] = (SHIFT * ln).astype(f32)
        tarange = np.arange(T)[None, :]
        valid = tarange < ln[:, None]                  # [16, T]
        selm = np.zeros((K, T, B), f32)
        jk = np.arange(K)[:, None, None]
        selm[:] = (tg.T[None] == jk) & valid.T[None]
        m["sel"] = np.ascontiguousarray(selm.reshape(K, T * B))
        counts = np.zeros((B, 144), f32)
        cntb = np.zeros((B, K), f32)
        for b in range(B):
            L = int(ln[b])
            prev = START
            for t in range(L):
                nx = int(tg[b, t])
                counts[b, nx * K + prev] += 1
                cntb[b, nx] += 1
                prev = nx
            counts[b, STOP * K + prev] += 1
        m["counts"] = counts
        m["cntb"] = cntb
        maps.append(m)
    return maps


def kernel(**inputs):
    from concourse.bass_utils import run_bass_kernel_spmd
    nc = get_program()
    maps = host_prep(inputs)
    res = run_bass_kernel_spmd(nc, maps, core_ids=list(range(NCORES)))
    out = np.concatenate([r["nll"] for r in res.results]).astype(np.float32)
    kernel.last_results = res
    return out


# revision 15
# speedup vs baseline: 1.0130x; 1.0130x over previous
"""BiLSTM-CRF NLL kernel for 8 TRN2 NeuronCores (v2).

Sharding: data-parallel over batch. B=128 split into 8 shards of 16
sentences; each core runs both LSTM directions, the fc projection, the
CRF forward pass and the gold-path score for its shard.

v2 design (vs baseline):
  - W_ih folded into the embedding table on the host:
    preW[v] = emb[v] @ W_ih^T + (b_ih + b_hh), bf16, rows permuted to
    [i|f|o|g] with the g block pre-scaled by 2 (tanh(g) = 2*sigmoid(2g)-1).
    The per-step input contribution is a single indirect row gather +
    16 PE transposes + one DVE copy per 8-step window per direction.
  - Gates accumulate in PSUM: identity-matmul injects the pre slice,
    then 64 bf16 W_hh matmuls accumulate on top (start=False). No
    separate pre-add on the elementwise chain.
  - Per-step elementwise (per dir): 2 sigmoids (gi, fo views), fused
    tanh(g) via tensor_scalar 2s-1, 4-5 DVE tensor ops, 1 tanh.
    Forward dir runs unmasked (post-length values unused); backward
    keeps masked state in cst/hcurb via copy_predicated.
  - CRF: stationary matrix augmented to [12,13] with an all-ones column
    so every step's matmul also yields the column sum (for renorm)
    for free; renormalization is applied two epochs late off the
    critical chain; per-step constant e^-2.5 damping (compensated by
    +2.5*len at the end) keeps magnitudes in f32 range.
"""

import os
import numpy as np
import ml_dtypes

import concourse.bass as bass
import concourse.bacc as bacc
import concourse.mybir as mybir
import concourse.tile as tile
from concourse.bass import AP
from concourse.masks import make_identity

F32 = mybir.dt.float32
BF16 = mybir.dt.bfloat16
I32 = mybir.dt.int32
U8 = mybir.dt.uint8
MUL = mybir.AluOpType.mult
ADD = mybir.AluOpType.add
SUB = mybir.AluOpType.subtract
X = mybir.AxisListType.X
SIG = mybir.ActivationFunctionType.Sigmoid
TANH = mybir.ActivationFunctionType.Tanh
EXP = mybir.ActivationFunctionType.Exp
LN = mybir.ActivationFunctionType.Ln

P = 128
B = 16            # batch per core
H = 512
G = 2048          # 4H
K = 12
START, STOP = 10, 11
R = 8             # CRF renorm epoch length
W = 8             # pre-gather window (steps per indirect gather)
NCORES = 8
SHIFT = 2.5       # per-step CRF damping exp(-SHIFT)

T = int(os.environ.get("BASS_LSTM_T", "256"))
SKIP = set(os.environ.get("BASS_SKIP", "").split(","))
NW = T // W       # number of gather windows
NJ = T // R - 1   # number of CRF renorm epochs with a recorded sum


def fv(t, off, pat):
    """Free-dim view of a contiguous [P, F] tile: keep partition pair, replace
    free dims with `pat` (list of [step, count]) at element offset `off`."""
    base = t[:] if not isinstance(t, AP) else t
    part = list(base.ap[0])
    return AP(base.tensor, base.offset + off, [part] + [list(p) for p in pat])


def build(nc):
    dirs = ("f", "b")
    dt = {}

    def din(name, shape, dtype):
        dt[name] = nc.dram_tensor(name, shape, dtype, kind="ExternalInput")
        return dt[name]

    for d in dirs:
        din(f"xw_{d}", [T * B], I32)
        din(f"preW_{d}", [30000, G], BF16)
        din(f"whhT_{d}", [H, G], BF16)
        din(f"h0T_{d}", [P, 64], BF16)
        din(f"c0T_{d}", [P, 64], F32)
        din(f"fcWT_{d}", [H, K], BF16)
    din("mask_b", [T, P, 64], U8)
    din("transT", [K, K], F32)
    din("trans", [K, K], F32)
    din("fcb", [K], F32)
    din("a0", [K, B], F32)
    din("msel", [K, T * B], F32)
    din("mprefix", [NJ * B], F32)
    din("lenc", [B], F32)
    din("sel", [K, T * B], F32)
    din("counts", [B, 144], F32)
    din("cntb", [B, K], F32)

    nll_o = nc.dram_tensor("nll", [B], F32, kind="ExternalOutput")
    demis_o = nc.dram_tensor("dbg_emis", [K, T * B], F32, kind="ExternalOutput")
    dlogz_o = nc.dram_tensor("dbg_logz", [B], F32, kind="ExternalOutput")
    dgold_o = nc.dram_tensor("dbg_gold", [B], F32, kind="ExternalOutput")
    dhs_o = None
    if os.environ.get("BASS_DBG"):
        dhs_o = {d: nc.dram_tensor(f"dbg_hs_{d}", [P, (T + 1) * 64], BF16,
                                   kind="ExternalOutput") for d in ("f", "b")}
    dg0_o = None
    if os.environ.get("BASS_DBG"):
        dg0_o = nc.dram_tensor("dbg_g0", [P, 512], F32, kind="ExternalOutput")
        dpre_o = nc.dram_tensor("dbg_pre0", [P, G], BF16, kind="ExternalOutput")

    scr16 = nc.dram_tensor("scr16", [B], F32)

    with tile.TileContext(nc) as tc:
        with tc.tile_pool(name="persist", bufs=1) as pp:
            whh = {d: pp.tile([P, 4 * 16 * P], BF16, name=f"whh{d}", tag=f"whh{d}")
                   for d in dirs}
            fcw = {d: pp.tile([P, 4 * K], BF16, name=f"fcw{d}", tag=f"fcw{d}") for d in dirs}
            hs = {d: pp.tile([P, (T + 1) * 64], BF16, name=f"hs{d}", tag=f"hs{d}")
                  for d in dirs}
            cst = {d: pp.tile([P, 64], F32, name=f"cst{d}", tag=f"c{d}") for d in dirs}
            hcurb = pp.tile([P, 64], BF16, tag="hcurb")
            identB = pp.tile([P, P], BF16, tag="identB")
            emisT = pp.tile([K, T * B], F32, tag="emisT")
            hist = pp.tile([K, T * B], F32, tag="hist")
            expem = pp.tile([K, T * B], F32, tag="expem")
            Sall = pp.tile([1, (NJ + 1) * B], F32, tag="Sall")
            idxall = {d: pp.tile([P, NW], I32, name=f"idxall{d}", tag=f"idxall{d}")
                      for d in dirs}

            make_identity(nc, identB[:])
            nc.gpsimd.memset(Sall[:], 1.0)
            for d in dirs:
                for k in range(4):
                    nc.gpsimd.dma_start(
                        whh[d][:, k * 16 * P:(k + 1) * 16 * P],
                        dt[f"whhT_{d}"].ap()[k * P:(k + 1) * P, :])
                    nc.gpsimd.dma_start(
                        fcw[d][:, k * K:(k + 1) * K],
                        dt[f"fcWT_{d}"].ap()[k * P:(k + 1) * P, :])
                nc.gpsimd.dma_start(hs[d][:, 0:64], dt[f"h0T_{d}"].ap()[:])
                nc.gpsimd.dma_start(cst[d][:], dt[f"c0T_{d}"].ap()[:])
                nc.gpsimd.dma_start(
                    idxall[d][:], AP(dt[f"xw_{d}"], 0, [[1, P], [P, NW]]))
            nc.gpsimd.dma_start(hcurb[:], dt["h0T_b"].ap()[:])

            # ---- recurrence with inlined pre-staging ----
            with tc.tile_pool(name="rec_sbuf", bufs=2) as rp, \
                 tc.tile_pool(name="stage_psum", bufs=1, space="PSUM") as stp, \
                 tc.tile_pool(name="gate_psum", bufs=3, space="PSUM") as gpp:

                prechW = {}
                maskch = None

                def stage(w, d):
                    rows = rp.tile([P, G], BF16, name=f"rows{d}", tag=f"rows{d}")
                    nc.gpsimd.indirect_dma_start(
                        out=rows[:], out_offset=None,
                        in_=dt[f"preW_{d}"].ap()[:],
                        in_offset=bass.IndirectOffsetOnAxis(
                            ap=idxall[d][:, w:w + 1], axis=0))
                    stg = stp.tile([P, G], BF16, name=f"stg{d}", tag="stg")
                    for m in range(16):
                        nc.tensor.transpose(
                            stg[:, m * P:(m + 1) * P], rows[:, m * P:(m + 1) * P],
                            identB[:])
                    pc = rp.tile([P, G], BF16, name=f"prech{d}", tag=f"prech{d}")
                    nc.vector.tensor_copy(pc[:], stg[:])
                    return pc

                def load_mask(w):
                    mk = rp.tile([P, W * 64], U8, tag="maskch")
                    nc.gpsimd.dma_start(
                        mk[:], AP(dt["mask_b"], w * W * P * 64,
                                  [[64, P], [P * 64, W], [1, 64]]))
                    return mk

                if "rec" not in SKIP:
                    for d in dirs:
                        prechW[d] = stage(0, d)
                    maskch = load_mask(0)
                nextprech = {}

                # m-order: g block first, then i, then f,o — lets sigma(g,i)
                # start while the f,o matmuls still stream.
                m_order = [12, 13, 14, 15, 0, 1, 2, 3, 4, 5, 6, 7, 8, 9, 10, 11]

                rec_range = range(0, T) if "rec" not in SKIP else range(0)
                for t in rec_range:
                    w, tl = t // W, t % W
                    if tl == 0 and w + 1 < NW:
                        for d in dirs:
                            nextprech[d] = stage(w + 1, d)
                        nextmask = load_mask(w + 1)
                    psd = {d: gpp.tile([P, 256], F32, name=f"gates{d}",
                                       tag=f"gates{d}") for d in dirs}
                    mkv = maskch[:, tl * 64:(tl + 1) * 64]
                    for d in dirs:
                        nc.tensor.matmul(
                            psd[d][:], identB[:],
                            fv(prechW[d], tl * B, [[P, 16], [1, B]]),
                            start=True, stop=False, skip_group_check=True)
                    for d in dirs:
                        for mi, m in enumerate(m_order):
                            for k in range(4):
                                if d == "f":
                                    rhs = hs[d][:, t * 64 + k * B: t * 64 + (k + 1) * B]
                                else:
                                    rhs = hcurb[:, k * B:(k + 1) * B]
                                nc.tensor.matmul(
                                    psd[d][:, m * B:(m + 1) * B],
                                    whh[d][:, (k * 16 + m) * P:(k * 16 + m + 1) * P],
                                    rhs, start=False,
                                    stop=(mi == 15 and k == 3),
                                    skip_group_check=True)
                    # elementwise, cross-direction interleaved so each in-order
                    # engine queue matches expected data-ready times.
                    sfd, ud, w2d, t1d, tcd = {}, {}, {}, {}, {}
                    for d in dirs:
                        sfd[d] = rp.tile([P, 256], F32, name=f"sifo{d}", tag=f"sifo{d}")
                        ud[d] = rp.tile([P, 64], F32, name=f"u{d}", tag=f"u{d}")
                        w2d[d] = rp.tile([P, 64], F32, name=f"w2{d}", tag=f"w2{d}")
                        t1d[d] = rp.tile([P, 64], F32, name=f"t1{d}", tag=f"t1{d}")
                        tcd[d] = rp.tile([P, 64], F32, name=f"tc{d}", tag=f"tc{d}")
                    cnb = rp.tile([P, 64], F32, tag="cnb")
                    sf = sfd["f"]
                    nc.scalar.activation(sfd["f"][:], psd["f"][:], SIG)      # Act
                    nc.vector.scalar_tensor_tensor(                          # DVE
                        ud["f"][:], sfd["f"][:, 192:256], 2.0, sfd["f"][:, 0:64],
                        op0=MUL, op1=MUL)
                    nc.vector.scalar_tensor_tensor(
                        w2d["f"][:], ud["f"][:], 0.0, sfd["f"][:, 0:64],
                        op0=SUB, op1=SUB)
                    nc.vector.scalar_tensor_tensor(
                        t1d["f"][:], cst["f"][:], 1.0, sfd["f"][:, 64:128],
                        op0=MUL, op1=MUL)
                    nc.vector.scalar_tensor_tensor(
                        cst["f"][:], t1d["f"][:], 0.0, w2d["f"][:], op0=ADD, op1=ADD)
                    nc.scalar.activation(sfd["b"][:], psd["b"][:], SIG)      # Act
                    nc.scalar.activation(tcd["f"][:], cst["f"][:], TANH)     # Act
                    nc.vector.scalar_tensor_tensor(                          # DVE
                        ud["b"][:], sfd["b"][:, 192:256], 2.0, sfd["b"][:, 0:64],
                        op0=MUL, op1=MUL)
                    nc.vector.scalar_tensor_tensor(
                        w2d["b"][:], ud["b"][:], 0.0, sfd["b"][:, 0:64],
                        op0=SUB, op1=SUB)
                    nc.vector.scalar_tensor_tensor(
                        t1d["b"][:], cst["b"][:], 1.0, sfd["b"][:, 64:128],
                        op0=MUL, op1=MUL)
                    hslot_f = hs["f"][:, (t + 1) * 64:(t + 2) * 64]
                    nc.vector.scalar_tensor_tensor(
                        hslot_f, sfd["f"][:, 128:192], 1.0, tcd["f"][:],
                        op0=MUL, op1=MUL)
                    nc.vector.scalar_tensor_tensor(
                        cnb[:], t1d["b"][:], 0.0, w2d["b"][:], op0=ADD, op1=ADD)
                    nc.vector.copy_predicated(cst["b"][:], mkv, cnb[:])
                    nc.scalar.activation(tcd["b"][:], cnb[:], TANH)          # Act
                    hslot_b = hs["b"][:, (t + 1) * 64:(t + 2) * 64]
                    nc.vector.scalar_tensor_tensor(                          # DVE
                        hslot_b, sfd["b"][:, 128:192], 1.0, tcd["b"][:],
                        op0=MUL, op1=MUL)
                    nc.vector.copy_predicated(hcurb[:], mkv, hslot_b)
                    if tl == W - 1 and w + 1 < NW:
                        maskch = nextmask
                        prechW = dict(nextprech)

            if dhs_o is not None:
                for d in dirs:
                    nc.gpsimd.dma_start(dhs_o[d].ap()[:], hs[d][:])

            # ---- fc + CRF (interleaved) ----
            with tc.tile_pool(name="crf_sbuf", bufs=2) as cp, \
                 tc.tile_pool(name="crf_persist", bufs=1) as cpr, \
                 tc.tile_pool(name="rs_pool", bufs=3) as rsp, \
                 tc.tile_pool(name="fc_psum", bufs=2, space="PSUM") as fpp, \
                 tc.tile_pool(name="crf_psum", bufs=2, space="PSUM") as cpp:
                # [12, 33] stationary: cols 0:12 = exp(trans)^T, col 32 = ones
                # (colsum lands on out partition 32 — partition reads must be
                # 32-aligned per the BIR verifier).
                etA = cpr.tile([K, 33], F32, tag="etA")
                transTs = cpr.tile([K, K], F32, tag="transTs")
                nc.gpsimd.dma_start(transTs[:], dt["transT"].ap()[:])
                nc.gpsimd.memset(etA[:], 0.0)
                nc.scalar.activation(etA[:, 0:K], transTs[:], EXP)
                nc.gpsimd.memset(etA[:, 32:33], 1.0)
                Estop = cpr.tile([K, 1], F32, tag="Estop")
                nc.scalar.activation(Estop[:], transTs[:, STOP:STOP + 1], EXP)
                fcbm = cpr.tile([K, 1], F32, tag="fcbm")
                nc.gpsimd.dma_start(fcbm[:], AP(dt["fcb"], 0, [[1, K], [1, 1]]))
                nc.vector.tensor_scalar(out=fcbm[:], in0=fcbm[:], scalar1=SHIFT,
                                        scalar2=None, op0=SUB)
                a0 = cpr.tile([K, B], F32, tag="a0")
                nc.gpsimd.dma_start(a0[:], dt["a0"].ap()[:])

                NCH = T * B // 512
                rsap = {}

                def fc_chunk(c):
                    psf = fpp.tile([K, 512], F32, tag="psf")
                    for d in dirs:
                        for k in range(4):
                            if d == "f":
                                rhs = fv(hs[d], (c * 32 + 1) * 64 + k * B,
                                         [[64, 32], [1, B]])
                            else:
                                rhs = fv(hs[d], (T - c * 32) * 64 + k * B,
                                         [[-64, 32], [1, B]])
                            nc.tensor.matmul(
                                psf[:], fcw[d][:, k * K:(k + 1) * K], rhs,
                                start=(d == "f" and k == 0),
                                stop=(d == "b" and k == 3))
                    nc.vector.tensor_copy(emisT[:, c * 512:(c + 1) * 512], psf[:])
                    nc.scalar.activation(expem[:, c * 512:(c + 1) * 512],
                                         emisT[:, c * 512:(c + 1) * 512], EXP,
                                         bias=fcbm[:, 0:1])

                for t in range(0 if "crf" not in SKIP else T, T):
                    if t % 32 == 0 and "fc" not in SKIP:
                        fc_chunk(t // 32)
                    doS = (t % R == 0 and t >= R)
                    doApply = (t % R == 0 and t >= 2 * R)
                    j = t // R - 1
                    for hh, (lo, hi) in enumerate(((0, 8), (8, B))):
                        psc = cpp.tile([33, 8], F32, tag=f"psc{hh}", name=f"psc{hh}")
                        if t == 0:
                            rhs = a0[:, lo:hi]
                        else:
                            rhs = hist[:, (t - 1) * B + lo:(t - 1) * B + hi]
                        nc.tensor.matmul(psc[:], etA[:], rhs,
                                         start=True, stop=True)
                        if doS:
                            nc.vector.tensor_copy(
                                Sall[:, j * B + lo:j * B + hi], psc[32:33, :])
                        nc.vector.tensor_tensor(
                            hist[:, t * B + lo:t * B + hi], psc[0:K, :],
                            expem[:, t * B + lo:t * B + hi], op=MUL)
                        if doApply:
                            nc.vector.tensor_tensor(
                                hist[:, t * B + lo:t * B + hi],
                                hist[:, t * B + lo:t * B + hi],
                                rsap[j - 1][:, lo:hi], op=MUL)
                    if doS:
                        rs1 = cp.tile([1, B], F32, tag="rs1")
                        nc.vector.reciprocal(rs1[:], Sall[:, j * B:(j + 1) * B])
                        ra = rsp.tile([K, B], F32, tag="rsap")
                        nc.gpsimd.partition_broadcast(ra[:], rs1[:])
                        rsap[j] = ra

                if "crf" not in SKIP:
                    # capture at t = len-1
                    mselb = cpr.tile([K, T * B], F32, tag="mselb")
                    nc.gpsimd.dma_start(mselb[:], dt["msel"].ap()[:])
                    nc.vector.tensor_tensor(hist[:], hist[:], mselb[:], op=MUL)
                    aend = cp.tile([K, B], F32, tag="aend")
                    nc.vector.tensor_reduce(aend[:], fv(hist, 0, [[1, B], [B, T]]),
                                            axis=X, op=ADD)
                    azs = cp.tile([K, B], F32, tag="azs")
                    nc.vector.tensor_scalar(out=azs[:], in0=aend[:],
                                            scalar1=Estop[:, 0:1], scalar2=None,
                                            op0=MUL)
                    psz = cpp.tile([33, B], F32, tag="psz", bufs=1)
                    nc.tensor.matmul(psz[:], etA[:], azs[:],
                                     start=True, stop=True)
                    logz0 = cp.tile([1, B], F32, tag="logz0")
                    nc.scalar.activation(logz0[:], psz[32:33, :], LN)
                    # renorm compensation: sum_j ln(S_j) * mprefix
                    lnS = cp.tile([1, NJ * B], F32, tag="lnS")
                    nc.scalar.activation(lnS[:], Sall[:, 0:NJ * B], LN)
                    mpf = cp.tile([1, NJ * B], F32, tag="mpf")
                    nc.gpsimd.dma_start(mpf[:], AP(dt["mprefix"], 0,
                                                   [[1, 1], [1, NJ * B]]))
                    nc.vector.tensor_tensor(lnS[:], lnS[:], mpf[:], op=MUL)
                    Lend = cp.tile([1, B], F32, tag="Lend")
                    nc.vector.tensor_reduce(Lend[:], fv(lnS, 0, [[1, B], [B, NJ]]),
                                            axis=X, op=ADD)
                    lencs = cp.tile([1, B], F32, tag="lencs")
                    nc.gpsimd.dma_start(lencs[:], AP(dt["lenc"], 0, [[1, 1], [1, B]]))
                    logzf = cp.tile([1, B], F32, tag="logzf")
                    nc.vector.tensor_tensor(logzf[:], logz0[:], Lend[:], op=ADD)
                    nc.vector.tensor_tensor(logzf[:], logzf[:], lencs[:], op=ADD)
                    nc.gpsimd.dma_start(AP(dlogz_o, 0, [[1, 1], [1, B]]), logzf[:])
                    nc.gpsimd.dma_start(demis_o.ap()[:], emisT[:])

                    # ---- gold score ----
                    tfl = cp.tile([1, 144], F32, tag="tfl")
                    nc.gpsimd.dma_start(tfl[:], AP(dt["trans"], 0, [[1, 1], [1, 144]]))
                    tfb = cp.tile([B, 144], F32, tag="tfb")
                    nc.gpsimd.partition_broadcast(tfb[:], tfl[:])
                    cnts = cp.tile([B, 144], F32, tag="cnts")
                    nc.gpsimd.dma_start(cnts[:], dt["counts"].ap()[:])
                    pr1 = cp.tile([B, 144], F32, tag="pr1")
                    nc.vector.tensor_tensor(pr1[:], cnts[:], tfb[:], op=MUL)
                    g1 = cp.tile([B, 1], F32, tag="g1")
                    nc.vector.tensor_reduce(g1[:], pr1[:], axis=X, op=ADD)
                    fcbr = cp.tile([1, K], F32, tag="fcbr")
                    nc.gpsimd.dma_start(fcbr[:], AP(dt["fcb"], 0, [[1, 1], [1, K]]))
                    fcbb = cp.tile([B, K], F32, tag="fcbb")
                    nc.gpsimd.partition_broadcast(fcbb[:], fcbr[:])
                    cntbs = cp.tile([B, K], F32, tag="cntbs")
                    nc.gpsimd.dma_start(cntbs[:], dt["cntb"].ap()[:])
                    pr2 = cp.tile([B, K], F32, tag="pr2")
                    nc.vector.tensor_tensor(pr2[:], cntbs[:], fcbb[:], op=MUL)
                    g2 = cp.tile([B, 1], F32, tag="g2")
                    nc.vector.tensor_reduce(g2[:], pr2[:], axis=X, op=ADD)
                    g12 = cp.tile([B, 1], F32, tag="g12")
                    nc.vector.tensor_tensor(g12[:], g1[:], g2[:], op=ADD)
                    nc.gpsimd.dma_start(AP(scr16, 0, [[1, B], [1, 1]]), g12[:])
                    g12r = cp.tile([1, B], F32, tag="g12r")
                    nc.gpsimd.dma_start(g12r[:], AP(scr16, 0, [[1, 1], [1, B]]))

                    selb = cpr.tile([K, T * B], F32, tag="selb")
                    nc.gpsimd.dma_start(selb[:], dt["sel"].ap()[:])
                    nc.vector.tensor_tensor(selb[:], emisT[:], selb[:], op=MUL)
                    g3 = cp.tile([K, B], F32, tag="g3")
                    nc.vector.tensor_reduce(g3[:], fv(selb, 0, [[1, B], [B, T]]),
                                            axis=X, op=ADD)
                    psg = cpp.tile([33, B], F32, tag="psg", bufs=1)
                    nc.tensor.matmul(psg[:], etA[:], g3[:],
                                     start=True, stop=True)
                    goldT = cp.tile([1, B], F32, tag="goldT")
                    nc.vector.tensor_tensor(goldT[:], g12r[:], psg[32:33, :], op=ADD)
                    nc.gpsimd.dma_start(AP(dgold_o, 0, [[1, 1], [1, B]]), goldT[:])
                    nllT = cp.tile([1, B], F32, tag="nllT")
                    nc.vector.tensor_tensor(nllT[:], logzf[:], goldT[:], op=SUB)
                    nc.gpsimd.dma_start(AP(nll_o, 0, [[1, 1], [1, B]]), nllT[:])
    return nc


_CACHE = {}


def get_program():
    if "nc" not in _CACHE:
        nc = bacc.Bacc("TRN2", target_bir_lowering=False, debug=False,
                       num_devices=NCORES)
        build(nc)
        nc.compile()
        _CACHE["nc"] = nc
    return _CACHE["nc"]


def perm_ifog(w):
    # [4H, ...] rows i,f,g,o -> i,f,o,g
    return np.concatenate([w[0:512], w[512:1024], w[1536:2048], w[1024:1536]], 0)


def host_prep(inputs):
    f32 = np.float32
    bf = ml_dtypes.bfloat16
    x = np.asarray(inputs["x"]).astype(np.int32)
    lengths = np.asarray(inputs["lengths"]).astype(np.int64)
    tags = np.asarray(inputs["tags"]).astype(np.int64)
    emb = np.asarray(inputs["embedding"], f32)
    trans = np.asarray(inputs["trans"], f32)
    fcW = np.asarray(inputs["fc_W"], f32)
    fcb = np.asarray(inputs["fc_b"], f32)
    h0 = np.asarray(inputs["h0"], f32)
    c0 = np.asarray(inputs["c0"], f32)

    preWd, whhTd = {}, {}
    for d in ("f", "b"):
        wih = perm_ifog(np.asarray(inputs[f"W_ih_{d}"], f32))
        whh = perm_ifog(np.asarray(inputs[f"W_hh_{d}"], f32))
        bi = perm_ifog(np.asarray(inputs[f"b_ih_{d}"], f32)[:, None])[:, 0]
        bh = perm_ifog(np.asarray(inputs[f"b_hh_{d}"], f32)[:, None])[:, 0]
        preW = emb @ wih.T + (bi + bh)[None, :]
        preW[:, 1536:2048] *= 2.0
        preWd[d] = preW.astype(bf)
        whhT = whh.T.copy()
        whhT[:, 1536:2048] *= 2.0
        whhTd[d] = whhT.astype(bf).copy()

    fcWT = {"f": fcW[:, :H].T.astype(bf).copy(), "b": fcW[:, H:].T.astype(bf).copy()}

    maps = []
    for c in range(NCORES):
        bs = slice(c * B, (c + 1) * B)
        xs = x[bs]            # [16, T]
        ln = lengths[bs]      # [16]
        tg = tags[bs]         # [16, T]
        m = {"trans": trans, "transT": trans.T.astype(f32).copy(), "fcb": fcb}
        for d in ("f", "b"):
            xt = xs.T if d == "f" else xs.T[::-1]      # [T, 16]
            m[f"xw_{d}"] = np.ascontiguousarray(xt).reshape(-1).astype(np.int32)
            m[f"preW_{d}"] = preWd[d]
            m[f"whhT_{d}"] = whhTd[d]
            m[f"fcWT_{d}"] = fcWT[d]
            di = 0 if d == "f" else 1
            h0T = h0[di, bs].T.reshape(4, P, B).transpose(1, 0, 2).reshape(P, 64)
            c0T = c0[di, bs].T.reshape(4, P, B).transpose(1, 0, 2).reshape(P, 64)
            m[f"h0T_{d}"] = h0T.astype(bf).copy()
            m[f"c0T_{d}"] = c0T.astype(f32).copy()
        # bwd mask: step s processes tau = T-1-s; valid iff tau < len
        tau = (T - 1 - np.arange(T))[:, None]          # [T, 1]
        mk = (tau < ln[None, :]).astype(f32)           # [T, 16]
        m["mask_b"] = np.broadcast_to(
            mk[:, None, None, :], (T, P, 4, B)).reshape(T, P, 64).astype(np.uint8).copy()
        a0 = np.zeros((K, B), f32); a0[START, :] = 1.0
        m["a0"] = a0
        msel = np.zeros((K, T, B), f32)
        msel[:, ln - 1, np.arange(B)] = 1.0
        m["msel"] = msel.reshape(K, T * B)
        # renorm j applied at step 8j+16; counted iff 8j+16 <= len-1
        jj = np.arange(NJ)[:, None]
        m["mprefix"] = ((R * jj + 2 * R) <= (ln[None, :] - 1)).astype(f32).reshape(-1)
        m["lenc"] = (SHIFT * ln).astype(f32)
        tarange = np.arange(T)[None, :]
        valid = tarange < ln[:, None]                  # [16, T]
        selm = np.zeros((K, T, B), f32)
        jk = np.arange(K)[:, None, None]
        selm[:] = (tg.T[None] == jk) & valid.T[None]
        m["sel"] = np.ascontiguousarray(selm.reshape(K, T * B))
        counts = np.zeros((B, 144), f32)
        cntb = np.zeros((B, K), f32)
        for b in range(B):
            L = int(ln[b])
            prev = START
            for t in range(L):
                nx = int(tg[b, t])
                counts[b, nx * K + prev] += 1
                cntb[b, nx] += 1
                prev = nx
            counts[b, STOP * K + prev] += 1
        m["counts"] = counts
        m["cntb"] = cntb
        maps.append(m)
    return maps


def kernel(**inputs):
    from concourse.bass_utils import run_bass_kernel_spmd
    nc = get_program()
    maps = host_prep(inputs)
    res = run_bass_kernel_spmd(nc, maps, core_ids=list(range(NCORES)))
    out = np.concatenate([r["nll"] for r in res.results]).astype(np.float32)
    kernel.last_results = res
    return out


# revision 16
# speedup vs baseline: 1.0139x; 1.0009x over previous
"""BiLSTM-CRF NLL kernel for 8 TRN2 NeuronCores (v2).

Sharding: data-parallel over batch. B=128 split into 8 shards of 16
sentences; each core runs both LSTM directions, the fc projection, the
CRF forward pass and the gold-path score for its shard.

v2 design (vs baseline):
  - W_ih folded into the embedding table on the host:
    preW[v] = emb[v] @ W_ih^T + (b_ih + b_hh), bf16, rows permuted to
    [i|f|o|g] with the g block pre-scaled by 2 (tanh(g) = 2*sigmoid(2g)-1).
    The per-step input contribution is a single indirect row gather +
    16 PE transposes + one DVE copy per 8-step window per direction.
  - Gates accumulate in PSUM: identity-matmul injects the pre slice,
    then 64 bf16 W_hh matmuls accumulate on top (start=False). No
    separate pre-add on the elementwise chain.
  - Per-step elementwise (per dir): 2 sigmoids (gi, fo views), fused
    tanh(g) via tensor_scalar 2s-1, 4-5 DVE tensor ops, 1 tanh.
    Forward dir runs unmasked (post-length values unused); backward
    keeps masked state in cst/hcurb via copy_predicated.
  - CRF: stationary matrix augmented to [12,13] with an all-ones column
    so every step's matmul also yields the column sum (for renorm)
    for free; renormalization is applied two epochs late off the
    critical chain; per-step constant e^-2.5 damping (compensated by
    +2.5*len at the end) keeps magnitudes in f32 range.
"""

import os
import numpy as np
import ml_dtypes

import concourse.bass as bass
import concourse.bacc as bacc
import concourse.mybir as mybir
import concourse.tile as tile
from concourse.bass import AP
from concourse.masks import make_identity

F32 = mybir.dt.float32
BF16 = mybir.dt.bfloat16
I32 = mybir.dt.int32
U8 = mybir.dt.uint8
MUL = mybir.AluOpType.mult
ADD = mybir.AluOpType.add
SUB = mybir.AluOpType.subtract
X = mybir.AxisListType.X
SIG = mybir.ActivationFunctionType.Sigmoid
TANH = mybir.ActivationFunctionType.Tanh
EXP = mybir.ActivationFunctionType.Exp
LN = mybir.ActivationFunctionType.Ln

P = 128
B = 16            # batch per core
H = 512
G = 2048          # 4H
K = 12
START, STOP = 10, 11
R = 8             # CRF renorm epoch length
W = 8             # pre-gather window (steps per indirect gather)
NCORES = 8
SHIFT = 2.5       # per-step CRF damping exp(-SHIFT)

T = int(os.environ.get("BASS_LSTM_T", "256"))
SKIP = set(os.environ.get("BASS_SKIP", "").split(","))
NW = T // W       # number of gather windows
NJ = T // R - 1   # number of CRF renorm epochs with a recorded sum


def fv(t, off, pat):
    """Free-dim view of a contiguous [P, F] tile: keep partition pair, replace
    free dims with `pat` (list of [step, count]) at element offset `off`."""
    base = t[:] if not isinstance(t, AP) else t
    part = list(base.ap[0])
    return AP(base.tensor, base.offset + off, [part] + [list(p) for p in pat])


def build(nc):
    dirs = ("f", "b")
    dt = {}

    def din(name, shape, dtype):
        dt[name] = nc.dram_tensor(name, shape, dtype, kind="ExternalInput")
        return dt[name]

    for d in dirs:
        din(f"xw_{d}", [T * B], I32)
        din(f"preW_{d}", [30000, G], BF16)
        din(f"whhT_{d}", [H, G], BF16)
        din(f"h0T_{d}", [P, 64], BF16)
        din(f"c0T_{d}", [P, 64], F32)
        din(f"fcWT_{d}", [H, K], BF16)
    din("mask_b", [T, P, 64], U8)
    din("transT", [K, K], F32)
    din("trans", [K, K], F32)
    din("fcb", [K], F32)
    din("a0", [K, B], F32)
    din("msel", [K, T * B], F32)
    din("mprefix", [NJ * B], F32)
    din("lenc", [B], F32)
    din("sel", [K, T * B], F32)
    din("counts", [B, 144], F32)
    din("cntb", [B, K], F32)

    nll_o = nc.dram_tensor("nll", [B], F32, kind="ExternalOutput")
    demis_o = nc.dram_tensor("dbg_emis", [K, T * B], F32, kind="ExternalOutput")
    dlogz_o = nc.dram_tensor("dbg_logz", [B], F32, kind="ExternalOutput")
    dgold_o = nc.dram_tensor("dbg_gold", [B], F32, kind="ExternalOutput")
    dhs_o = None
    if os.environ.get("BASS_DBG"):
        dhs_o = {d: nc.dram_tensor(f"dbg_hs_{d}", [P, (T + 1) * 64], BF16,
                                   kind="ExternalOutput") for d in ("f", "b")}
    dg0_o = None
    if os.environ.get("BASS_DBG"):
        dg0_o = nc.dram_tensor("dbg_g0", [P, 512], F32, kind="ExternalOutput")
        dpre_o = nc.dram_tensor("dbg_pre0", [P, G], BF16, kind="ExternalOutput")

    scr16 = nc.dram_tensor("scr16", [B], F32)

    with tile.TileContext(nc) as tc:
        with tc.tile_pool(name="persist", bufs=1) as pp:
            whh = {d: pp.tile([P, 4 * 16 * P], BF16, name=f"whh{d}", tag=f"whh{d}")
                   for d in dirs}
            fcw = {d: pp.tile([P, 4 * K], BF16, name=f"fcw{d}", tag=f"fcw{d}") for d in dirs}
            hs = {d: pp.tile([P, (T + 1) * 64], BF16, name=f"hs{d}", tag=f"hs{d}")
                  for d in dirs}
            cst = {d: pp.tile([P, 64], F32, name=f"cst{d}", tag=f"c{d}") for d in dirs}
            hcurb = pp.tile([P, 64], BF16, tag="hcurb")
            identB = pp.tile([P, P], BF16, tag="identB")
            emisT = pp.tile([K, T * B], F32, tag="emisT")
            hist = pp.tile([K, T * B], F32, tag="hist")
            expem = pp.tile([K, T * B], F32, tag="expem")
            Sall = pp.tile([1, (NJ + 1) * B], F32, tag="Sall")
            idxall = {d: pp.tile([P, NW], I32, name=f"idxall{d}", tag=f"idxall{d}")
                      for d in dirs}

            make_identity(nc, identB[:])
            nc.gpsimd.memset(Sall[:], 1.0)
            for d in dirs:
                for k in range(4):
                    nc.gpsimd.dma_start(
                        whh[d][:, k * 16 * P:(k + 1) * 16 * P],
                        dt[f"whhT_{d}"].ap()[k * P:(k + 1) * P, :])
                    nc.gpsimd.dma_start(
                        fcw[d][:, k * K:(k + 1) * K],
                        dt[f"fcWT_{d}"].ap()[k * P:(k + 1) * P, :])
                nc.gpsimd.dma_start(hs[d][:, 0:64], dt[f"h0T_{d}"].ap()[:])
                nc.gpsimd.dma_start(cst[d][:], dt[f"c0T_{d}"].ap()[:])
                nc.gpsimd.dma_start(
                    idxall[d][:], AP(dt[f"xw_{d}"], 0, [[1, P], [P, NW]]))
            nc.gpsimd.dma_start(hcurb[:], dt["h0T_b"].ap()[:])

            # ---- recurrence with inlined pre-staging ----
            with tc.tile_pool(name="rec_sbuf", bufs=2) as rp, \
                 tc.tile_pool(name="stage_psum", bufs=1, space="PSUM") as stp, \
                 tc.tile_pool(name="gate_psum", bufs=3, space="PSUM") as gpp:

                prechW = {}
                maskch = None

                def stage(w, d):
                    rows = rp.tile([P, G], BF16, name=f"rows{d}", tag=f"rows{d}")
                    nc.gpsimd.indirect_dma_start(
                        out=rows[:], out_offset=None,
                        in_=dt[f"preW_{d}"].ap()[:],
                        in_offset=bass.IndirectOffsetOnAxis(
                            ap=idxall[d][:, w:w + 1], axis=0))
                    stg = stp.tile([P, G], BF16, name=f"stg{d}", tag="stg")
                    for m in range(16):
                        nc.tensor.transpose(
                            stg[:, m * P:(m + 1) * P], rows[:, m * P:(m + 1) * P],
                            identB[:])
                    pc = rp.tile([P, G], BF16, name=f"prech{d}", tag=f"prech{d}")
                    nc.vector.tensor_copy(pc[:], stg[:])
                    return pc

                def load_mask(w):
                    mk = rp.tile([P, W * 64], U8, tag="maskch")
                    nc.gpsimd.dma_start(
                        mk[:], AP(dt["mask_b"], w * W * P * 64,
                                  [[64, P], [P * 64, W], [1, 64]]))
                    return mk

                if "rec" not in SKIP:
                    for d in dirs:
                        prechW[d] = stage(0, d)
                    maskch = load_mask(0)
                nextprech = {}

                # m-order: g block first, then i, then f,o — lets sigma(g,i)
                # start while the f,o matmuls still stream.
                m_order = [12, 13, 14, 15, 0, 1, 2, 3, 4, 5, 6, 7, 8, 9, 10, 11]

                rec_range = range(0, T) if "rec" not in SKIP else range(0)
                for t in rec_range:
                    w, tl = t // W, t % W
                    if tl == 0 and w + 1 < NW:
                        for d in dirs:
                            nextprech[d] = stage(w + 1, d)
                        nextmask = load_mask(w + 1)
                    psd = {d: gpp.tile([P, 256], F32, name=f"gates{d}",
                                       tag=f"gates{d}") for d in dirs}
                    mkv = maskch[:, tl * 64:(tl + 1) * 64]
                    for d in dirs:
                        nc.tensor.matmul(
                            psd[d][:], identB[:],
                            fv(prechW[d], tl * B, [[P, 16], [1, B]]),
                            start=True, stop=False, skip_group_check=True)
                    for d in dirs:
                        for mi, m in enumerate(m_order):
                            for k in range(4):
                                if d == "f":
                                    rhs = hs[d][:, t * 64 + k * B: t * 64 + (k + 1) * B]
                                else:
                                    rhs = hcurb[:, k * B:(k + 1) * B]
                                nc.tensor.matmul(
                                    psd[d][:, m * B:(m + 1) * B],
                                    whh[d][:, (k * 16 + m) * P:(k * 16 + m + 1) * P],
                                    rhs, start=False,
                                    stop=(mi == 15 and k == 3),
                                    skip_group_check=True)
                    # elementwise, cross-direction interleaved so each in-order
                    # engine queue matches expected data-ready times.
                    sfd, ud, w2d, t1d, tcd = {}, {}, {}, {}, {}
                    for d in dirs:
                        sfd[d] = rp.tile([P, 256], F32, name=f"sifo{d}", tag=f"sifo{d}")
                        ud[d] = rp.tile([P, 64], F32, name=f"u{d}", tag=f"u{d}")
                        w2d[d] = rp.tile([P, 64], F32, name=f"w2{d}", tag=f"w2{d}")
                        t1d[d] = rp.tile([P, 64], F32, name=f"t1{d}", tag=f"t1{d}")
                        tcd[d] = rp.tile([P, 64], F32, name=f"tc{d}", tag=f"tc{d}")
                    cnb = rp.tile([P, 64], F32, tag="cnb")
                    sf = sfd["f"]
                    nc.scalar.activation(sfd["f"][:], psd["f"][:], SIG)      # Act
                    # w2' = (sg - 0.5) * sigma_i ; cn = 2*w2' + t1
                    nc.vector.scalar_tensor_tensor(                          # DVE
                        w2d["f"][:], sfd["f"][:, 192:256], 0.5, sfd["f"][:, 0:64],
                        op0=SUB, op1=MUL)
                    nc.vector.scalar_tensor_tensor(
                        t1d["f"][:], cst["f"][:], 1.0, sfd["f"][:, 64:128],
                        op0=MUL, op1=MUL)
                    nc.vector.scalar_tensor_tensor(
                        cst["f"][:], w2d["f"][:], 2.0, t1d["f"][:], op0=MUL, op1=ADD)
                    nc.scalar.activation(sfd["b"][:], psd["b"][:], SIG)      # Act
                    nc.scalar.activation(tcd["f"][:], cst["f"][:], TANH)     # Act
                    nc.vector.scalar_tensor_tensor(                          # DVE
                        w2d["b"][:], sfd["b"][:, 192:256], 0.5, sfd["b"][:, 0:64],
                        op0=SUB, op1=MUL)
                    nc.vector.scalar_tensor_tensor(
                        t1d["b"][:], cst["b"][:], 1.0, sfd["b"][:, 64:128],
                        op0=MUL, op1=MUL)
                    hslot_f = hs["f"][:, (t + 1) * 64:(t + 2) * 64]
                    nc.vector.scalar_tensor_tensor(
                        hslot_f, sfd["f"][:, 128:192], 1.0, tcd["f"][:],
                        op0=MUL, op1=MUL)
                    nc.vector.scalar_tensor_tensor(
                        cnb[:], w2d["b"][:], 2.0, t1d["b"][:], op0=MUL, op1=ADD)
                    nc.vector.copy_predicated(cst["b"][:], mkv, cnb[:])
                    nc.scalar.activation(tcd["b"][:], cnb[:], TANH)          # Act
                    hslot_b = hs["b"][:, (t + 1) * 64:(t + 2) * 64]
                    nc.vector.scalar_tensor_tensor(                          # DVE
                        hslot_b, sfd["b"][:, 128:192], 1.0, tcd["b"][:],
                        op0=MUL, op1=MUL)
                    nc.vector.copy_predicated(hcurb[:], mkv, hslot_b)
                    if tl == W - 1 and w + 1 < NW:
                        maskch = nextmask
                        prechW = dict(nextprech)

            if dhs_o is not None:
                for d in dirs:
                    nc.gpsimd.dma_start(dhs_o[d].ap()[:], hs[d][:])

            # ---- fc + CRF (interleaved) ----
            with tc.tile_pool(name="crf_sbuf", bufs=2) as cp, \
                 tc.tile_pool(name="crf_persist", bufs=1) as cpr, \
                 tc.tile_pool(name="rs_pool", bufs=3) as rsp, \
                 tc.tile_pool(name="fc_psum", bufs=2, space="PSUM") as fpp, \
                 tc.tile_pool(name="crf_psum", bufs=2, space="PSUM") as cpp:
                # [12, 33] stationary: cols 0:12 = exp(trans)^T, col 32 = ones
                # (colsum lands on out partition 32 — partition reads must be
                # 32-aligned per the BIR verifier).
                etA = cpr.tile([K, 33], F32, tag="etA")
                transTs = cpr.tile([K, K], F32, tag="transTs")
                nc.gpsimd.dma_start(transTs[:], dt["transT"].ap()[:])
                nc.gpsimd.memset(etA[:], 0.0)
                nc.scalar.activation(etA[:, 0:K], transTs[:], EXP)
                nc.gpsimd.memset(etA[:, 32:33], 1.0)
                Estop = cpr.tile([K, 1], F32, tag="Estop")
                nc.scalar.activation(Estop[:], transTs[:, STOP:STOP + 1], EXP)
                fcbm = cpr.tile([K, 1], F32, tag="fcbm")
                nc.gpsimd.dma_start(fcbm[:], AP(dt["fcb"], 0, [[1, K], [1, 1]]))
                nc.vector.tensor_scalar(out=fcbm[:], in0=fcbm[:], scalar1=SHIFT,
                                        scalar2=None, op0=SUB)
                a0 = cpr.tile([K, B], F32, tag="a0")
                nc.gpsimd.dma_start(a0[:], dt["a0"].ap()[:])

                NCH = T * B // 512
                rsap = {}

                def fc_chunk(c):
                    psf = fpp.tile([K, 512], F32, tag="psf")
                    for d in dirs:
                        for k in range(4):
                            if d == "f":
                                rhs = fv(hs[d], (c * 32 + 1) * 64 + k * B,
                                         [[64, 32], [1, B]])
                            else:
                                rhs = fv(hs[d], (T - c * 32) * 64 + k * B,
                                         [[-64, 32], [1, B]])
                            nc.tensor.matmul(
                                psf[:], fcw[d][:, k * K:(k + 1) * K], rhs,
                                start=(d == "f" and k == 0),
                                stop=(d == "b" and k == 3))
                    nc.vector.tensor_copy(emisT[:, c * 512:(c + 1) * 512], psf[:])
                    nc.scalar.activation(expem[:, c * 512:(c + 1) * 512],
                                         emisT[:, c * 512:(c + 1) * 512], EXP,
                                         bias=fcbm[:, 0:1])

                for t in range(0 if "crf" not in SKIP else T, T):
                    if t % 32 == 0 and "fc" not in SKIP:
                        fc_chunk(t // 32)
                    doS = (t % R == 0 and t >= R)
                    doApply = (t % R == 0 and t >= 2 * R)
                    j = t // R - 1
                    for hh, (lo, hi) in enumerate(((0, 8), (8, B))):
                        psc = cpp.tile([33, 8], F32, tag=f"psc{hh}", name=f"psc{hh}")
                        if t == 0:
                            rhs = a0[:, lo:hi]
                        else:
                            rhs = hist[:, (t - 1) * B + lo:(t - 1) * B + hi]
                        nc.tensor.matmul(psc[:], etA[:], rhs,
                                         start=True, stop=True)
                        if doS:
                            nc.vector.tensor_copy(
                                Sall[:, j * B + lo:j * B + hi], psc[32:33, :])
                        nc.vector.tensor_tensor(
                            hist[:, t * B + lo:t * B + hi], psc[0:K, :],
                            expem[:, t * B + lo:t * B + hi], op=MUL)
                        if doApply:
                            nc.vector.tensor_tensor(
                                hist[:, t * B + lo:t * B + hi],
                                hist[:, t * B + lo:t * B + hi],
                                rsap[j - 1][:, lo:hi], op=MUL)
                    if doS:
                        rs1 = cp.tile([1, B], F32, tag="rs1")
                        nc.vector.reciprocal(rs1[:], Sall[:, j * B:(j + 1) * B])
                        ra = rsp.tile([K, B], F32, tag="rsap")
                        nc.gpsimd.partition_broadcast(ra[:], rs1[:])
                        rsap[j] = ra

                if "crf" not in SKIP:
                    # capture at t = len-1
                    mselb = cpr.tile([K, T * B], F32, tag="mselb")
                    nc.gpsimd.dma_start(mselb[:], dt["msel"].ap()[:])
                    nc.vector.tensor_tensor(hist[:], hist[:], mselb[:], op=MUL)
                    aend = cp.tile([K, B], F32, tag="aend")
                    nc.vector.tensor_reduce(aend[:], fv(hist, 0, [[1, B], [B, T]]),
                                            axis=X, op=ADD)
                    azs = cp.tile([K, B], F32, tag="azs")
                    nc.vector.tensor_scalar(out=azs[:], in0=aend[:],
                                            scalar1=Estop[:, 0:1], scalar2=None,
                                            op0=MUL)
                    psz = cpp.tile([33, B], F32, tag="psz", bufs=1)
                    nc.tensor.matmul(psz[:], etA[:], azs[:],
                                     start=True, stop=True)
                    logz0 = cp.tile([1, B], F32, tag="logz0")
                    nc.scalar.activation(logz0[:], psz[32:33, :], LN)
                    # renorm compensation: sum_j ln(S_j) * mprefix
                    lnS = cp.tile([1, NJ * B], F32, tag="lnS")
                    nc.scalar.activation(lnS[:], Sall[:, 0:NJ * B], LN)
                    mpf = cp.tile([1, NJ * B], F32, tag="mpf")
                    nc.gpsimd.dma_start(mpf[:], AP(dt["mprefix"], 0,
                                                   [[1, 1], [1, NJ * B]]))
                    nc.vector.tensor_tensor(lnS[:], lnS[:], mpf[:], op=MUL)
                    Lend = cp.tile([1, B], F32, tag="Lend")
                    nc.vector.tensor_reduce(Lend[:], fv(lnS, 0, [[1, B], [B, NJ]]),
                                            axis=X, op=ADD)
                    lencs = cp.tile([1, B], F32, tag="lencs")
                    nc.gpsimd.dma_start(lencs[:], AP(dt["lenc"], 0, [[1, 1], [1, B]]))
                    logzf = cp.tile([1, B], F32, tag="logzf")
                    nc.vector.tensor_tensor(logzf[:], logz0[:], Lend[:], op=ADD)
                    nc.vector.tensor_tensor(logzf[:], logzf[:], lencs[:], op=ADD)
                    nc.gpsimd.dma_start(AP(dlogz_o, 0, [[1, 1], [1, B]]), logzf[:])
                    nc.gpsimd.dma_start(demis_o.ap()[:], emisT[:])

                    # ---- gold score ----
                    tfl = cp.tile([1, 144], F32, tag="tfl")
                    nc.gpsimd.dma_start(tfl[:], AP(dt["trans"], 0, [[1, 1], [1, 144]]))
                    tfb = cp.tile([B, 144], F32, tag="tfb")
                    nc.gpsimd.partition_broadcast(tfb[:], tfl[:])
                    cnts = cp.tile([B, 144], F32, tag="cnts")
                    nc.gpsimd.dma_start(cnts[:], dt["counts"].ap()[:])
                    pr1 = cp.tile([B, 144], F32, tag="pr1")
                    nc.vector.tensor_tensor(pr1[:], cnts[:], tfb[:], op=MUL)
                    g1 = cp.tile([B, 1], F32, tag="g1")
                    nc.vector.tensor_reduce(g1[:], pr1[:], axis=X, op=ADD)
                    fcbr = cp.tile([1, K], F32, tag="fcbr")
                    nc.gpsimd.dma_start(fcbr[:], AP(dt["fcb"], 0, [[1, 1], [1, K]]))
                    fcbb = cp.tile([B, K], F32, tag="fcbb")
                    nc.gpsimd.partition_broadcast(fcbb[:], fcbr[:])
                    cntbs = cp.tile([B, K], F32, tag="cntbs")
                    nc.gpsimd.dma_start(cntbs[:], dt["cntb"].ap()[:])
                    pr2 = cp.tile([B, K], F32, tag="pr2")
                    nc.vector.tensor_tensor(pr2[:], cntbs[:], fcbb[:], op=MUL)
                    g2 = cp.tile([B, 1], F32, tag="g2")
                    nc.vector.tensor_reduce(g2[:], pr2[:], axis=X, op=ADD)
                    g12 = cp.tile([B, 1], F32, tag="g12")
                    nc.vector.tensor_tensor(g12[:], g1[:], g2[:], op=ADD)
                    nc.gpsimd.dma_start(AP(scr16, 0, [[1, B], [1, 1]]), g12[:])
                    g12r = cp.tile([1, B], F32, tag="g12r")
                    nc.gpsimd.dma_start(g12r[:], AP(scr16, 0, [[1, 1], [1, B]]))

                    selb = cpr.tile([K, T * B], F32, tag="selb")
                    nc.gpsimd.dma_start(selb[:], dt["sel"].ap()[:])
                    nc.vector.tensor_tensor(selb[:], emisT[:], selb[:], op=MUL)
                    g3 = cp.tile([K, B], F32, tag="g3")
                    nc.vector.tensor_reduce(g3[:], fv(selb, 0, [[1, B], [B, T]]),
                                            axis=X, op=ADD)
                    psg = cpp.tile([33, B], F32, tag="psg", bufs=1)
                    nc.tensor.matmul(psg[:], etA[:], g3[:],
                                     start=True, stop=True)
                    goldT = cp.tile([1, B], F32, tag="goldT")
                    nc.vector.tensor_tensor(goldT[:], g12r[:], psg[32:33, :], op=ADD)
                    nc.gpsimd.dma_start(AP(dgold_o, 0, [[1, 1], [1, B]]), goldT[:])
                    nllT = cp.tile([1, B], F32, tag="nllT")
                    nc.vector.tensor_tensor(nllT[:], logzf[:], goldT[:], op=SUB)
                    nc.gpsimd.dma_start(AP(nll_o, 0, [[1, 1], [1, B]]), nllT[:])
    return nc


_CACHE = {}


def get_program():
    if "nc" not in _CACHE:
        nc = bacc.Bacc("TRN2", target_bir_lowering=False, debug=False,
                       num_devices=NCORES)
        build(nc)
        nc.compile()
        _CACHE["nc"] = nc
    return _CACHE["nc"]


def perm_ifog(w):
    # [4H, ...] rows i,f,g,o -> i,f,o,g
    return np.concatenate([w[0:512], w[512:1024], w[1536:2048], w[1024:1536]], 0)


def host_prep(inputs):
    f32 = np.float32
    bf = ml_dtypes.bfloat16
    x = np.asarray(inputs["x"]).astype(np.int32)
    lengths = np.asarray(inputs["lengths"]).astype(np.int64)
    tags = np.asarray(inputs["tags"]).astype(np.int64)
    emb = np.asarray(inputs["embedding"], f32)
    trans = np.asarray(inputs["trans"], f32)
    fcW = np.asarray(inputs["fc_W"], f32)
    fcb = np.asarray(inputs["fc_b"], f32)
    h0 = np.asarray(inputs["h0"], f32)
    c0 = np.asarray(inputs["c0"], f32)

    preWd, whhTd = {}, {}
    for d in ("f", "b"):
        wih = perm_ifog(np.asarray(inputs[f"W_ih_{d}"], f32))
        whh = perm_ifog(np.asarray(inputs[f"W_hh_{d}"], f32))
        bi = perm_ifog(np.asarray(inputs[f"b_ih_{d}"], f32)[:, None])[:, 0]
        bh = perm_ifog(np.asarray(inputs[f"b_hh_{d}"], f32)[:, None])[:, 0]
        preW = emb @ wih.T + (bi + bh)[None, :]
        preW[:, 1536:2048] *= 2.0
        preWd[d] = preW.astype(bf)
        whhT = whh.T.copy()
        whhT[:, 1536:2048] *= 2.0
        whhTd[d] = whhT.astype(bf).copy()

    fcWT = {"f": fcW[:, :H].T.astype(bf).copy(), "b": fcW[:, H:].T.astype(bf).copy()}

    maps = []
    for c in range(NCORES):
        bs = slice(c * B, (c + 1) * B)
        xs = x[bs]            # [16, T]
        ln = lengths[bs]      # [16]
        tg = tags[bs]         # [16, T]
        m = {"trans": trans, "transT": trans.T.astype(f32).copy(), "fcb": fcb}
        for d in ("f", "b"):
            xt = xs.T if d == "f" else xs.T[::-1]      # [T, 16]
            m[f"xw_{d}"] = np.ascontiguousarray(xt).reshape(-1).astype(np.int32)
            m[f"preW_{d}"] = preWd[d]
            m[f"whhT_{d}"] = whhTd[d]
            m[f"fcWT_{d}"] = fcWT[d]
            di = 0 if d == "f" else 1
            h0T = h0[di, bs].T.reshape(4, P, B).transpose(1, 0, 2).reshape(P, 64)
            c0T = c0[di, bs].T.reshape(4, P, B).transpose(1, 0, 2).reshape(P, 64)
            m[f"h0T_{d}"] = h0T.astype(bf).copy()
            m[f"c0T_{d}"] = c0T.astype(f32).copy()
        # bwd mask: step s processes tau = T-1-s; valid iff tau < len
        tau = (T - 1 - np.arange(T))[:, None]          # [T, 1]
        mk = (tau < ln[None, :]).astype(f32)           # [T, 16]
        m["mask_b"] = np.broadcast_to(
            mk[:, None, None, :], (T, P, 4, B)).reshape(T, P, 64).astype(np.uint8).copy()
        a0 = np.zeros((K, B), f32); a0[START, :] = 1.0
        m["a0"] = a0
        msel = np.zeros((K, T, B), f32)
        msel[:, ln - 1, np.arange(B)] = 1.0
        m["msel"] = msel.reshape(K, T * B)
        # renorm j applied at step 8j+16; counted iff 8j+16 <= len-1
        jj = np.arange(NJ)[:, None]
        m["mprefix"] = ((R * jj + 2 * R) <= (ln[None, :] - 1)).astype(f32).reshape(-1)
        m["lenc"] = (SHIFT * ln).astype(f32)
        tarange = np.arange(T)[None, :]
        valid = tarange < ln[:, None]                  # [16, T]
        selm = np.zeros((K, T, B), f32)
        jk = np.arange(K)[:, None, None]
        selm[:] = (tg.T[None] == jk) & valid.T[None]
        m["sel"] = np.ascontiguousarray(selm.reshape(K, T * B))
        counts = np.zeros((B, 144), f32)
        cntb = np.zeros((B, K), f32)
        for b in range(B):
            L = int(ln[b])
            prev = START
            for t in range(L):
                nx = int(tg[b, t])
                counts[b, nx * K + prev] += 1
                cntb[b, nx] += 1
                prev = nx
            counts[b, STOP * K + prev] += 1
        m["counts"] = counts
        m["cntb"] = cntb
        maps.append(m)
    return maps


def kernel(**inputs):
    from concourse.bass_utils import run_bass_kernel_spmd
    nc = get_program()
    maps = host_prep(inputs)
    res = run_bass_kernel_spmd(nc, maps, core_ids=list(range(NCORES)))
    out = np.concatenate([r["nll"] for r in res.results]).astype(np.float32)
    kernel.last_results = res
    return out


# revision 17
# speedup vs baseline: 1.0141x; 1.0002x over previous
"""BiLSTM-CRF NLL kernel for 8 TRN2 NeuronCores (v2).

Sharding: data-parallel over batch. B=128 split into 8 shards of 16
sentences; each core runs both LSTM directions, the fc projection, the
CRF forward pass and the gold-path score for its shard.

v2 design (vs baseline):
  - W_ih folded into the embedding table on the host:
    preW[v] = emb[v] @ W_ih^T + (b_ih + b_hh), bf16, rows permuted to
    [i|f|o|g] with the g block pre-scaled by 2 (tanh(g) = 2*sigmoid(2g)-1).
    The per-step input contribution is a single indirect row gather +
    16 PE transposes + one DVE copy per 8-step window per direction.
  - Gates accumulate in PSUM: identity-matmul injects the pre slice,
    then 64 bf16 W_hh matmuls accumulate on top (start=False). No
    separate pre-add on the elementwise chain.
  - Per-step elementwise (per dir): 2 sigmoids (gi, fo views), fused
    tanh(g) via tensor_scalar 2s-1, 4-5 DVE tensor ops, 1 tanh.
    Forward dir runs unmasked (post-length values unused); backward
    keeps masked state in cst/hcurb via copy_predicated.
  - CRF: stationary matrix augmented to [12,13] with an all-ones column
    so every step's matmul also yields the column sum (for renorm)
    for free; renormalization is applied two epochs late off the
    critical chain; per-step constant e^-2.5 damping (compensated by
    +2.5*len at the end) keeps magnitudes in f32 range.
"""

import os
import numpy as np
import ml_dtypes

import concourse.bass as bass
import concourse.bacc as bacc
import concourse.mybir as mybir
import concourse.tile as tile
from concourse.bass import AP
from concourse.masks import make_identity

F32 = mybir.dt.float32
BF16 = mybir.dt.bfloat16
I32 = mybir.dt.int32
U8 = mybir.dt.uint8
MUL = mybir.AluOpType.mult
ADD = mybir.AluOpType.add
SUB = mybir.AluOpType.subtract
X = mybir.AxisListType.X
SIG = mybir.ActivationFunctionType.Sigmoid
TANH = mybir.ActivationFunctionType.Tanh
EXP = mybir.ActivationFunctionType.Exp
LN = mybir.ActivationFunctionType.Ln

P = 128
B = 16            # batch per core
H = 512
G = 2048          # 4H
K = 12
START, STOP = 10, 11
R = 8             # CRF renorm epoch length
W = 8             # pre-gather window (steps per indirect gather)
NCORES = 8
SHIFT = 2.5       # per-step CRF damping exp(-SHIFT)

T = int(os.environ.get("BASS_LSTM_T", "256"))
SKIP = set(os.environ.get("BASS_SKIP", "").split(","))
NW = T // W       # number of gather windows
NJ = T // R - 1   # number of CRF renorm epochs with a recorded sum


def fv(t, off, pat):
    """Free-dim view of a contiguous [P, F] tile: keep partition pair, replace
    free dims with `pat` (list of [step, count]) at element offset `off`."""
    base = t[:] if not isinstance(t, AP) else t
    part = list(base.ap[0])
    return AP(base.tensor, base.offset + off, [part] + [list(p) for p in pat])


def build(nc):
    dirs = ("f", "b")
    dt = {}

    def din(name, shape, dtype):
        dt[name] = nc.dram_tensor(name, shape, dtype, kind="ExternalInput")
        return dt[name]

    for d in dirs:
        din(f"xw_{d}", [T * B], I32)
        din(f"preW_{d}", [30000, G], BF16)
        din(f"whhT_{d}", [H, G], BF16)
        din(f"h0T_{d}", [P, 64], BF16)
        din(f"c0T_{d}", [P, 64], F32)
        din(f"fcWT_{d}", [H, K], BF16)
    din("mask_b", [T, P, 64], U8)
    din("transT", [K, K], F32)
    din("trans", [K, K], F32)
    din("fcb", [K], F32)
    din("a0", [K, B], F32)
    din("msel", [K, T * B], F32)
    din("mprefix", [NJ * B], F32)
    din("lenc", [B], F32)
    din("sel", [K, T * B], F32)
    din("counts", [B, 144], F32)
    din("cntb", [B, K], F32)

    nll_o = nc.dram_tensor("nll", [B], F32, kind="ExternalOutput")
    demis_o = nc.dram_tensor("dbg_emis", [K, T * B], F32, kind="ExternalOutput")
    dlogz_o = nc.dram_tensor("dbg_logz", [B], F32, kind="ExternalOutput")
    dgold_o = nc.dram_tensor("dbg_gold", [B], F32, kind="ExternalOutput")
    dhs_o = None
    if os.environ.get("BASS_DBG"):
        dhs_o = {d: nc.dram_tensor(f"dbg_hs_{d}", [P, (T + 1) * 64], BF16,
                                   kind="ExternalOutput") for d in ("f", "b")}
    dg0_o = None
    if os.environ.get("BASS_DBG"):
        dg0_o = nc.dram_tensor("dbg_g0", [P, 512], F32, kind="ExternalOutput")
        dpre_o = nc.dram_tensor("dbg_pre0", [P, G], BF16, kind="ExternalOutput")

    scr16 = nc.dram_tensor("scr16", [B], F32)

    with tile.TileContext(nc) as tc:
        with tc.tile_pool(name="persist", bufs=1) as pp:
            whh = {d: pp.tile([P, 4 * 16 * P], BF16, name=f"whh{d}", tag=f"whh{d}")
                   for d in dirs}
            fcw = {d: pp.tile([P, 4 * K], BF16, name=f"fcw{d}", tag=f"fcw{d}") for d in dirs}
            hs = {d: pp.tile([P, (T + 1) * 64], BF16, name=f"hs{d}", tag=f"hs{d}")
                  for d in dirs}
            cst = {d: pp.tile([P, 64], F32, name=f"cst{d}", tag=f"c{d}") for d in dirs}
            hcurb = pp.tile([P, 64], BF16, tag="hcurb")
            identB = pp.tile([P, P], BF16, tag="identB")
            emisT = pp.tile([K, T * B], F32, tag="emisT")
            hist = pp.tile([K, T * B], F32, tag="hist")
            expem = pp.tile([K, T * B], F32, tag="expem")
            Sall = pp.tile([1, (NJ + 1) * B], F32, tag="Sall")
            idxall = {d: pp.tile([P, NW], I32, name=f"idxall{d}", tag=f"idxall{d}")
                      for d in dirs}

            make_identity(nc, identB[:])
            nc.gpsimd.memset(Sall[:], 1.0)
            for d in dirs:
                for k in range(4):
                    nc.gpsimd.dma_start(
                        whh[d][:, k * 16 * P:(k + 1) * 16 * P],
                        dt[f"whhT_{d}"].ap()[k * P:(k + 1) * P, :])
                    nc.gpsimd.dma_start(
                        fcw[d][:, k * K:(k + 1) * K],
                        dt[f"fcWT_{d}"].ap()[k * P:(k + 1) * P, :])
                nc.gpsimd.dma_start(hs[d][:, 0:64], dt[f"h0T_{d}"].ap()[:])
                nc.gpsimd.dma_start(cst[d][:], dt[f"c0T_{d}"].ap()[:])
                nc.gpsimd.dma_start(
                    idxall[d][:], AP(dt[f"xw_{d}"], 0, [[1, P], [P, NW]]))
            nc.gpsimd.dma_start(hcurb[:], dt["h0T_b"].ap()[:])

            # ---- recurrence with inlined pre-staging ----
            with tc.tile_pool(name="rec_sbuf", bufs=2) as rp, \
                 tc.tile_pool(name="stage_psum", bufs=1, space="PSUM") as stp, \
                 tc.tile_pool(name="gate_psum", bufs=3, space="PSUM") as gpp:

                prechW = {}
                maskch = None

                rowsbuf = {}
                stgbuf = {}

                def stage_gather(w, d):
                    rows = rp.tile([P, G], BF16, name=f"rows{d}", tag=f"rows{d}",
                                   bufs=2)
                    nc.gpsimd.indirect_dma_start(
                        out=rows[:], out_offset=None,
                        in_=dt[f"preW_{d}"].ap()[:],
                        in_offset=bass.IndirectOffsetOnAxis(
                            ap=idxall[d][:, w:w + 1], axis=0))
                    rowsbuf[d] = rows

                def stage_tr(d, half):
                    if half == 0:
                        stgbuf[d] = stp.tile([P, G], BF16, name=f"stg{d}", tag="stg")
                    stg, rows = stgbuf[d], rowsbuf[d]
                    for m in range(8 * half, 8 * half + 8):
                        nc.tensor.transpose(
                            stg[:, m * P:(m + 1) * P], rows[:, m * P:(m + 1) * P],
                            identB[:])

                def stage_copy(d):
                    pc = rp.tile([P, G], BF16, name=f"prech{d}", tag=f"prech{d}",
                                 bufs=3)
                    nc.vector.tensor_copy(pc[:], stgbuf[d][:])
                    return pc

                def stage(w, d):
                    stage_gather(w, d)
                    stage_tr(d, 0)
                    stage_tr(d, 1)
                    return stage_copy(d)

                def load_mask(w):
                    mk = rp.tile([P, W * 64], U8, tag="maskch")
                    nc.gpsimd.dma_start(
                        mk[:], AP(dt["mask_b"], w * W * P * 64,
                                  [[64, P], [P * 64, W], [1, 64]]))
                    return mk

                if "rec" not in SKIP:
                    for d in dirs:
                        prechW[d] = stage(0, d)
                    nextprech = {d: stage(1, d) for d in dirs}
                    maskch = load_mask(0)
                    nextmask = load_mask(1)

                # m-order: g block first, then i, then f,o — lets sigma(g,i)
                # start while the f,o matmuls still stream.
                m_order = [12, 13, 14, 15, 0, 1, 2, 3, 4, 5, 6, 7, 8, 9, 10, 11]

                rec_range = range(0, T) if "rec" not in SKIP else range(0)
                farprech = {}
                for t in rec_range:
                    w, tl = t // W, t % W
                    prep = w + 2 < NW
                    if tl == 0 and prep:
                        for d in dirs:
                            stage_gather(w + 2, d)
                        farmask = load_mask(w + 2)
                    elif tl == 2 and prep:
                        stage_tr("f", 0)
                    elif tl == 3 and prep:
                        stage_tr("f", 1)
                    elif tl == 4 and prep:
                        farprech["f"] = stage_copy("f")
                        stage_tr("b", 0)
                    elif tl == 5 and prep:
                        stage_tr("b", 1)
                    elif tl == 6 and prep:
                        farprech["b"] = stage_copy("b")
                    psd = {d: gpp.tile([P, 256], F32, name=f"gates{d}",
                                       tag=f"gates{d}") for d in dirs}
                    mkv = maskch[:, tl * 64:(tl + 1) * 64]
                    for d in dirs:
                        nc.tensor.matmul(
                            psd[d][:], identB[:],
                            fv(prechW[d], tl * B, [[P, 16], [1, B]]),
                            start=True, stop=False, skip_group_check=True)
                    for d in dirs:
                        for mi, m in enumerate(m_order):
                            for k in range(4):
                                if d == "f":
                                    rhs = hs[d][:, t * 64 + k * B: t * 64 + (k + 1) * B]
                                else:
                                    rhs = hcurb[:, k * B:(k + 1) * B]
                                nc.tensor.matmul(
                                    psd[d][:, m * B:(m + 1) * B],
                                    whh[d][:, (k * 16 + m) * P:(k * 16 + m + 1) * P],
                                    rhs, start=False,
                                    stop=(mi == 15 and k == 3),
                                    skip_group_check=True)
                    # elementwise, cross-direction interleaved so each in-order
                    # engine queue matches expected data-ready times.
                    sfd, ud, w2d, t1d, tcd = {}, {}, {}, {}, {}
                    for d in dirs:
                        sfd[d] = rp.tile([P, 256], F32, name=f"sifo{d}", tag=f"sifo{d}")
                        ud[d] = rp.tile([P, 64], F32, name=f"u{d}", tag=f"u{d}")
                        w2d[d] = rp.tile([P, 64], F32, name=f"w2{d}", tag=f"w2{d}")
                        t1d[d] = rp.tile([P, 64], F32, name=f"t1{d}", tag=f"t1{d}")
                        tcd[d] = rp.tile([P, 64], F32, name=f"tc{d}", tag=f"tc{d}")
                    cnb = rp.tile([P, 64], F32, tag="cnb")
                    sf = sfd["f"]
                    nc.scalar.activation(sfd["f"][:], psd["f"][:], SIG)      # Act
                    # w2' = (sg - 0.5) * sigma_i ; cn = 2*w2' + t1
                    nc.vector.scalar_tensor_tensor(                          # DVE
                        w2d["f"][:], sfd["f"][:, 192:256], 0.5, sfd["f"][:, 0:64],
                        op0=SUB, op1=MUL)
                    nc.vector.scalar_tensor_tensor(
                        t1d["f"][:], cst["f"][:], 1.0, sfd["f"][:, 64:128],
                        op0=MUL, op1=MUL)
                    nc.vector.scalar_tensor_tensor(
                        cst["f"][:], w2d["f"][:], 2.0, t1d["f"][:], op0=MUL, op1=ADD)
                    nc.scalar.activation(sfd["b"][:], psd["b"][:], SIG)      # Act
                    nc.scalar.activation(tcd["f"][:], cst["f"][:], TANH)     # Act
                    nc.vector.scalar_tensor_tensor(                          # DVE
                        w2d["b"][:], sfd["b"][:, 192:256], 0.5, sfd["b"][:, 0:64],
                        op0=SUB, op1=MUL)
                    nc.vector.scalar_tensor_tensor(
                        t1d["b"][:], cst["b"][:], 1.0, sfd["b"][:, 64:128],
                        op0=MUL, op1=MUL)
                    hslot_f = hs["f"][:, (t + 1) * 64:(t + 2) * 64]
                    nc.vector.scalar_tensor_tensor(
                        hslot_f, sfd["f"][:, 128:192], 1.0, tcd["f"][:],
                        op0=MUL, op1=MUL)
                    nc.vector.scalar_tensor_tensor(
                        cnb[:], w2d["b"][:], 2.0, t1d["b"][:], op0=MUL, op1=ADD)
                    nc.vector.copy_predicated(cst["b"][:], mkv, cnb[:])
                    nc.scalar.activation(tcd["b"][:], cnb[:], TANH)          # Act
                    hslot_b = hs["b"][:, (t + 1) * 64:(t + 2) * 64]
                    nc.vector.scalar_tensor_tensor(                          # DVE
                        hslot_b, sfd["b"][:, 128:192], 1.0, tcd["b"][:],
                        op0=MUL, op1=MUL)
                    nc.vector.copy_predicated(hcurb[:], mkv, hslot_b)
                    if tl == W - 1 and w + 1 < NW:
                        maskch = nextmask
                        prechW = dict(nextprech)
                        if w + 2 < NW:
                            nextmask = farmask
                            nextprech = dict(farprech)

            if dhs_o is not None:
                for d in dirs:
                    nc.gpsimd.dma_start(dhs_o[d].ap()[:], hs[d][:])

            # ---- fc + CRF (interleaved) ----
            with tc.tile_pool(name="crf_sbuf", bufs=2) as cp, \
                 tc.tile_pool(name="crf_persist", bufs=1) as cpr, \
                 tc.tile_pool(name="rs_pool", bufs=3) as rsp, \
                 tc.tile_pool(name="fc_psum", bufs=2, space="PSUM") as fpp, \
                 tc.tile_pool(name="crf_psum", bufs=2, space="PSUM") as cpp:
                # [12, 33] stationary: cols 0:12 = exp(trans)^T, col 32 = ones
                # (colsum lands on out partition 32 — partition reads must be
                # 32-aligned per the BIR verifier).
                etA = cpr.tile([K, 33], F32, tag="etA")
                transTs = cpr.tile([K, K], F32, tag="transTs")
                nc.gpsimd.dma_start(transTs[:], dt["transT"].ap()[:])
                nc.gpsimd.memset(etA[:], 0.0)
                nc.scalar.activation(etA[:, 0:K], transTs[:], EXP)
                nc.gpsimd.memset(etA[:, 32:33], 1.0)
                Estop = cpr.tile([K, 1], F32, tag="Estop")
                nc.scalar.activation(Estop[:], transTs[:, STOP:STOP + 1], EXP)
                fcbm = cpr.tile([K, 1], F32, tag="fcbm")
                nc.gpsimd.dma_start(fcbm[:], AP(dt["fcb"], 0, [[1, K], [1, 1]]))
                nc.vector.tensor_scalar(out=fcbm[:], in0=fcbm[:], scalar1=SHIFT,
                                        scalar2=None, op0=SUB)
                a0 = cpr.tile([K, B], F32, tag="a0")
                nc.gpsimd.dma_start(a0[:], dt["a0"].ap()[:])

                NCH = T * B // 512
                rsap = {}

                def fc_chunk(c):
                    psf = fpp.tile([K, 512], F32, tag="psf")
                    for d in dirs:
                        for k in range(4):
                            if d == "f":
                                rhs = fv(hs[d], (c * 32 + 1) * 64 + k * B,
                                         [[64, 32], [1, B]])
                            else:
                                rhs = fv(hs[d], (T - c * 32) * 64 + k * B,
                                         [[-64, 32], [1, B]])
                            nc.tensor.matmul(
                                psf[:], fcw[d][:, k * K:(k + 1) * K], rhs,
                                start=(d == "f" and k == 0),
                                stop=(d == "b" and k == 3))
                    nc.vector.tensor_copy(emisT[:, c * 512:(c + 1) * 512], psf[:])
                    nc.scalar.activation(expem[:, c * 512:(c + 1) * 512],
                                         emisT[:, c * 512:(c + 1) * 512], EXP,
                                         bias=fcbm[:, 0:1])

                for t in range(0 if "crf" not in SKIP else T, T):
                    if t % 32 == 0 and "fc" not in SKIP:
                        fc_chunk(t // 32)
                    doS = (t % R == 0 and t >= R)
                    doApply = (t % R == 0 and t >= 2 * R)
                    j = t // R - 1
                    for hh, (lo, hi) in enumerate(((0, 8), (8, B))):
                        psc = cpp.tile([33, 8], F32, tag=f"psc{hh}", name=f"psc{hh}")
                        if t == 0:
                            rhs = a0[:, lo:hi]
                        else:
                            rhs = hist[:, (t - 1) * B + lo:(t - 1) * B + hi]
                        nc.tensor.matmul(psc[:], etA[:], rhs,
                                         start=True, stop=True)
                        if doS:
                            nc.vector.tensor_copy(
                                Sall[:, j * B + lo:j * B + hi], psc[32:33, :])
                        nc.vector.tensor_tensor(
                            hist[:, t * B + lo:t * B + hi], psc[0:K, :],
                            expem[:, t * B + lo:t * B + hi], op=MUL)
                        if doApply:
                            nc.vector.tensor_tensor(
                                hist[:, t * B + lo:t * B + hi],
                                hist[:, t * B + lo:t * B + hi],
                                rsap[j - 1][:, lo:hi], op=MUL)
                    if doS:
                        rs1 = cp.tile([1, B], F32, tag="rs1")
                        nc.vector.reciprocal(rs1[:], Sall[:, j * B:(j + 1) * B])
                        ra = rsp.tile([K, B], F32, tag="rsap")
                        nc.gpsimd.partition_broadcast(ra[:], rs1[:])
                        rsap[j] = ra

                if "crf" not in SKIP:
                    # capture at t = len-1
                    mselb = cpr.tile([K, T * B], F32, tag="mselb")
                    nc.gpsimd.dma_start(mselb[:], dt["msel"].ap()[:])
                    nc.vector.tensor_tensor(hist[:], hist[:], mselb[:], op=MUL)
                    aend = cp.tile([K, B], F32, tag="aend")
                    nc.vector.tensor_reduce(aend[:], fv(hist, 0, [[1, B], [B, T]]),
                                            axis=X, op=ADD)
                    azs = cp.tile([K, B], F32, tag="azs")
                    nc.vector.tensor_scalar(out=azs[:], in0=aend[:],
                                            scalar1=Estop[:, 0:1], scalar2=None,
                                            op0=MUL)
                    psz = cpp.tile([33, B], F32, tag="psz", bufs=1)
                    nc.tensor.matmul(psz[:], etA[:], azs[:],
                                     start=True, stop=True)
                    logz0 = cp.tile([1, B], F32, tag="logz0")
                    nc.scalar.activation(logz0[:], psz[32:33, :], LN)
                    # renorm compensation: sum_j ln(S_j) * mprefix
                    lnS = cp.tile([1, NJ * B], F32, tag="lnS")
                    nc.scalar.activation(lnS[:], Sall[:, 0:NJ * B], LN)
                    mpf = cp.tile([1, NJ * B], F32, tag="mpf")
                    nc.gpsimd.dma_start(mpf[:], AP(dt["mprefix"], 0,
                                                   [[1, 1], [1, NJ * B]]))
                    nc.vector.tensor_tensor(lnS[:], lnS[:], mpf[:], op=MUL)
                    Lend = cp.tile([1, B], F32, tag="Lend")
                    nc.vector.tensor_reduce(Lend[:], fv(lnS, 0, [[1, B], [B, NJ]]),
                                            axis=X, op=ADD)
                    lencs = cp.tile([1, B], F32, tag="lencs")
                    nc.gpsimd.dma_start(lencs[:], AP(dt["lenc"], 0, [[1, 1], [1, B]]))
                    logzf = cp.tile([1, B], F32, tag="logzf")
                    nc.vector.tensor_tensor(logzf[:], logz0[:], Lend[:], op=ADD)
                    nc.vector.tensor_tensor(logzf[:], logzf[:], lencs[:], op=ADD)
                    nc.gpsimd.dma_start(AP(dlogz_o, 0, [[1, 1], [1, B]]), logzf[:])
                    nc.gpsimd.dma_start(demis_o.ap()[:], emisT[:])

                    # ---- gold score ----
                    tfl = cp.tile([1, 144], F32, tag="tfl")
                    nc.gpsimd.dma_start(tfl[:], AP(dt["trans"], 0, [[1, 1], [1, 144]]))
                    tfb = cp.tile([B, 144], F32, tag="tfb")
                    nc.gpsimd.partition_broadcast(tfb[:], tfl[:])
                    cnts = cp.tile([B, 144], F32, tag="cnts")
                    nc.gpsimd.dma_start(cnts[:], dt["counts"].ap()[:])
                    pr1 = cp.tile([B, 144], F32, tag="pr1")
                    nc.vector.tensor_tensor(pr1[:], cnts[:], tfb[:], op=MUL)
                    g1 = cp.tile([B, 1], F32, tag="g1")
                    nc.vector.tensor_reduce(g1[:], pr1[:], axis=X, op=ADD)
                    fcbr = cp.tile([1, K], F32, tag="fcbr")
                    nc.gpsimd.dma_start(fcbr[:], AP(dt["fcb"], 0, [[1, 1], [1, K]]))
                    fcbb = cp.tile([B, K], F32, tag="fcbb")
                    nc.gpsimd.partition_broadcast(fcbb[:], fcbr[:])
                    cntbs = cp.tile([B, K], F32, tag="cntbs")
                    nc.gpsimd.dma_start(cntbs[:], dt["cntb"].ap()[:])
                    pr2 = cp.tile([B, K], F32, tag="pr2")
                    nc.vector.tensor_tensor(pr2[:], cntbs[:], fcbb[:], op=MUL)
                    g2 = cp.tile([B, 1], F32, tag="g2")
                    nc.vector.tensor_reduce(g2[:], pr2[:], axis=X, op=ADD)
                    g12 = cp.tile([B, 1], F32, tag="g12")
                    nc.vector.tensor_tensor(g12[:], g1[:], g2[:], op=ADD)
                    nc.gpsimd.dma_start(AP(scr16, 0, [[1, B], [1, 1]]), g12[:])
                    g12r = cp.tile([1, B], F32, tag="g12r")
                    nc.gpsimd.dma_start(g12r[:], AP(scr16, 0, [[1, 1], [1, B]]))

                    selb = cpr.tile([K, T * B], F32, tag="selb")
                    nc.gpsimd.dma_start(selb[:], dt["sel"].ap()[:])
                    nc.vector.tensor_tensor(selb[:], emisT[:], selb[:], op=MUL)
                    g3 = cp.tile([K, B], F32, tag="g3")
                    nc.vector.tensor_reduce(g3[:], fv(selb, 0, [[1, B], [B, T]]),
                                            axis=X, op=ADD)
                    psg = cpp.tile([33, B], F32, tag="psg", bufs=1)
                    nc.tensor.matmul(psg[:], etA[:], g3[:],
                                     start=True, stop=True)
                    goldT = cp.tile([1, B], F32, tag="goldT")
                    nc.vector.tensor_tensor(goldT[:], g12r[:], psg[32:33, :], op=ADD)
                    nc.gpsimd.dma_start(AP(dgold_o, 0, [[1, 1], [1, B]]), goldT[:])
                    nllT = cp.tile([1, B], F32, tag="nllT")
                    nc.vector.tensor_tensor(nllT[:], logzf[:], goldT[:], op=SUB)
                    nc.gpsimd.dma_start(AP(nll_o, 0, [[1, 1], [1, B]]), nllT[:])
    return nc


_CACHE = {}


def get_program():
    if "nc" not in _CACHE:
        nc = bacc.Bacc("TRN2", target_bir_lowering=False, debug=False,
                       num_devices=NCORES)
        build(nc)
        nc.compile()
        _CACHE["nc"] = nc
    return _CACHE["nc"]


def perm_ifog(w):
    # [4H, ...] rows i,f,g,o -> i,f,o,g
    return np.concatenate([w[0:512], w[512:1024], w[1536:2048], w[1024:1536]], 0)


def host_prep(inputs):
    f32 = np.float32
    bf = ml_dtypes.bfloat16
    x = np.asarray(inputs["x"]).astype(np.int32)
    lengths = np.asarray(inputs["lengths"]).astype(np.int64)
    tags = np.asarray(inputs["tags"]).astype(np.int64)
    emb = np.asarray(inputs["embedding"], f32)
    trans = np.asarray(inputs["trans"], f32)
    fcW = np.asarray(inputs["fc_W"], f32)
    fcb = np.asarray(inputs["fc_b"], f32)
    h0 = np.asarray(inputs["h0"], f32)
    c0 = np.asarray(inputs["c0"], f32)

    preWd, whhTd = {}, {}
    for d in ("f", "b"):
        wih = perm_ifog(np.asarray(inputs[f"W_ih_{d}"], f32))
        whh = perm_ifog(np.asarray(inputs[f"W_hh_{d}"], f32))
        bi = perm_ifog(np.asarray(inputs[f"b_ih_{d}"], f32)[:, None])[:, 0]
        bh = perm_ifog(np.asarray(inputs[f"b_hh_{d}"], f32)[:, None])[:, 0]
        preW = emb @ wih.T + (bi + bh)[None, :]
        preW[:, 1536:2048] *= 2.0
        preWd[d] = preW.astype(bf)
        whhT = whh.T.copy()
        whhT[:, 1536:2048] *= 2.0
        whhTd[d] = whhT.astype(bf).copy()

    fcWT = {"f": fcW[:, :H].T.astype(bf).copy(), "b": fcW[:, H:].T.astype(bf).copy()}

    maps = []
    for c in range(NCORES):
        bs = slice(c * B, (c + 1) * B)
        xs = x[bs]            # [16, T]
        ln = lengths[bs]      # [16]
        tg = tags[bs]         # [16, T]
        m = {"trans": trans, "transT": trans.T.astype(f32).copy(), "fcb": fcb}
        for d in ("f", "b"):
            xt = xs.T if d == "f" else xs.T[::-1]      # [T, 16]
            m[f"xw_{d}"] = np.ascontiguousarray(xt).reshape(-1).astype(np.int32)
            m[f"preW_{d}"] = preWd[d]
            m[f"whhT_{d}"] = whhTd[d]
            m[f"fcWT_{d}"] = fcWT[d]
            di = 0 if d == "f" else 1
            h0T = h0[di, bs].T.reshape(4, P, B).transpose(1, 0, 2).reshape(P, 64)
            c0T = c0[di, bs].T.reshape(4, P, B).transpose(1, 0, 2).reshape(P, 64)
            m[f"h0T_{d}"] = h0T.astype(bf).copy()
            m[f"c0T_{d}"] = c0T.astype(f32).copy()
        # bwd mask: step s processes tau = T-1-s; valid iff tau < len
        tau = (T - 1 - np.arange(T))[:, None]          # [T, 1]
        mk = (tau < ln[None, :]).astype(f32)           # [T, 16]
        m["mask_b"] = np.broadcast_to(
            mk[:, None, None, :], (T, P, 4, B)).reshape(T, P, 64).astype(np.uint8).copy()
        a0 = np.zeros((K, B), f32); a0[START, :] = 1.0
        m["a0"] = a0
        msel = np.zeros((K, T, B), f32)
        msel[:, ln - 1, np.arange(B)] = 1.0
        m["msel"] = msel.reshape(K, T * B)
        # renorm j applied at step 8j+16; counted iff 8j+16 <= len-1
        jj = np.arange(NJ)[:, None]
        m["mprefix"] = ((R * jj + 2 * R) <= (ln[None, :] - 1)).astype(f32).reshape(-1)
        m["lenc"] = (SHIFT * ln).astype(f32)
        tarange = np.arange(T)[None, :]
        valid = tarange < ln[:, None]                  # [16, T]
        selm = np.zeros((K, T, B), f32)
        jk = np.arange(K)[:, None, None]
        selm[:] = (tg.T[None] == jk) & valid.T[None]
        m["sel"] = np.ascontiguousarray(selm.reshape(K, T * B))
        counts = np.zeros((B, 144), f32)
        cntb = np.zeros((B, K), f32)
        for b in range(B):
            L = int(ln[b])
            prev = START
            for t in range(L):
                nx = int(tg[b, t])
                counts[b, nx * K + prev] += 1
                cntb[b, nx] += 1
                prev = nx
            counts[b, STOP * K + prev] += 1
        m["counts"] = counts
        m["cntb"] = cntb
        maps.append(m)
    return maps


def kernel(**inputs):
    from concourse.bass_utils import run_bass_kernel_spmd
    nc = get_program()
    maps = host_prep(inputs)
    res = run_bass_kernel_spmd(nc, maps, core_ids=list(range(NCORES)))
    out = np.concatenate([r["nll"] for r in res.results]).astype(np.float32)
    kernel.last_results = res
    return out
